# revision 63
# baseline (speedup 1.0000x reference)
"""GCN (2x GCNConv + graph-layernorm + prelu + mean-pool + MLP head) on 8 trn2 cores.

Strategy (dst-sharded graph parallel):
  - nodes (and their incoming edges) sharded 8 ways by dst; weights replicated.
  - per-edge gather of source features via gpsimd dma_gather (fp16 table rows),
    segment-sum via one-hot selection matmuls on the TensorEngine (PSUM
    accumulate, fp16 accumulators in SBUF across src chunks).
  - L1 aggregates the 2-channel scaled positions; its graph-layernorm stats
    come from closed-form moments of s = dinv*agg (sum + 2x2 second moment),
    so no stats pass over x1 is needed; affine+prelu fold into per-tile
    Activation ops right out of PSUM (x1 kept fp16, transposed layout).
  - h2 = x1' @ W2 tables are AllGathered in fp16 as TWO half-shard
    collectives so the first overlaps the rest of h2 and the L2 gathers of
    its half; edges to a core's own nodes gather the local gshard halves
    without waiting for any collective (6-chunk layout, host-remapped rows).
  - the x2 epilogue (dinv scale, +b2, square/sum stats) runs per dst tile so
    it pipelines into the L2 aggregation tail; ln2 via AllReduce of scalars.
  - per-graph mean-pool partials combined via fp16 AllReduce; MLP head
    computed redundantly on every core.
All floating point compute happens on device; the host only shards/sorts/pads
integer index metadata and re-lays-out inputs.
"""

import numpy as np

P = 128


def _cfg_tiny():
    return dict(
        N=1900, E0=8000, G=128, IN_C=2, HID=256, OUT=16,
        NCORES=8, SH=256, CH=512, NCHUNK=4, CALL_TILES=4, QUEUES=1,
    )


def _cfg_full():
    return dict(
        N=100000, E0=3200000, G=512, IN_C=2, HID=256, OUT=16,
        NCORES=8, SH=12544, CH=32768, NCHUNK=4, CALL_TILES=32, QUEUES=4,
    )


# ----------------------------------------------------------------- host prep

def _wrap_idx(ix):
    """dma_gather idx layout: idx i -> [i%16 + 16k, i//16] for all k (replicated)."""
    m = ix.reshape(-1, 16).T
    return np.tile(m, (8, 1)).astype(np.int16)


def _edge_meta(cfg, srcs_pc, dsts_pc, rowmap, bounds):
    """Bucket per-core edges into (chunk, dst-tile) cells under a src->row map.

    bounds: chunk boundaries in the (possibly per-core) row space.
    Returns dict with tiles/CALLS (SPMD-uniform) and per-core wrapped idx +
    dst one-hot column streams."""
    c = cfg
    NCORES, SH = c["NCORES"], c["SH"]
    NT = SH // P
    bounds = np.asarray(bounds, np.int64)
    NCH = len(bounds) - 1
    percore = []
    counts = np.zeros((NCORES, NCH, NT), np.int64)
    for ci in range(NCORES):
        row = rowmap(ci, srcs_pc[ci])
        d = dsts_pc[ci]
        chunk = np.searchsorted(bounds, row, side="right") - 1
        row = row - bounds[chunk]
        o = np.lexsort((d, chunk))
        row, d, chunk = row[o], d[o], chunk[o]
        cnt = np.bincount(chunk * NT + d // P, minlength=NCH * NT).reshape(NCH, NT)
        counts[ci] = cnt
        percore.append((row, d, cnt))

    tiles = np.maximum(1, (counts.max(axis=0) + P - 1) // P)   # [NCH, NT]
    tiles_chunk = tiles.sum(axis=1)                            # [NCH]
    TOT = int(tiles.sum())

    idx_streams, dst_streams = [], []
    for ci in range(NCORES):
        row, d, cnt = percore[ci]
        idxs = np.zeros(TOT * P, np.int16)
        dcol = np.full(TOT * P, 999.0, np.float32)
        off = 0
        eoff = np.concatenate([[0], np.cumsum(cnt.ravel())])
        for ch in range(NCH):
            for t in range(NT):
                n = cnt[ch, t]
                e0 = eoff[ch * NT + t]
                slots = tiles[ch, t] * P
                idxs[off:off + n] = row[e0:e0 + n].astype(np.int16)
                dcol[off:off + n] = (d[e0:e0 + n] - t * P).astype(np.float32)
                off += slots
        assert off == TOT * P
        idx_streams.append(idxs)
        dst_streams.append(dcol)

    CALLS = []   # list of (chunk, ntiles) in stream order
    for ch in range(NCH):
        rem = int(tiles_chunk[ch])
        while rem > 0:
            k = min(c["CALL_TILES"], rem)
            CALLS.append((ch, k))
            rem -= k
    idx_wrapped = []
    for ci in range(NCORES):
        stream = idx_streams[ci]
        parts, off = [], 0
        for (_ch, k) in CALLS:
            parts.append(_wrap_idx(stream[off:off + k * P]))
            off += k * P
        idx_wrapped.append(np.concatenate(parts, axis=1))      # [128, TOT*8]
    dst_cols = [ds.reshape(TOT, P).T.copy() for ds in dst_streams]  # [128, TOT]
    return dict(TOT=TOT, tiles=tiles, CALLS=CALLS, idx=idx_wrapped,
                dstc=dst_cols)


def host_prep(cfg, pos, edge_index, batch):
    c = cfg
    N, E0, G, SH = c["N"], c["E0"], c["G"], c["SH"]
    NCH = c["NCHUNK"]
    NCORES = c["NCORES"]
    NPAD = SH * NCORES
    NT = SH // P                      # dst tiles per core
    CH = NPAD // NCH                  # exact chunking
    HSH = SH // 2
    src = np.concatenate([edge_index[0], np.arange(N, dtype=np.int64)]).astype(np.int64)
    dst = np.concatenate([edge_index[1], np.arange(N, dtype=np.int64)]).astype(np.int64)
    deg = np.bincount(dst, minlength=NPAD).astype(np.float32)

    so = np.argsort(dst, kind="stable")
    dsts = dst[so]
    srcs = src[so]
    bounds = np.searchsorted(dsts, np.arange(NCORES + 1) * SH)
    srcs_pc = [srcs[bounds[ci]:bounds[ci + 1]] for ci in range(NCORES)]
    dsts_pc = [dsts[bounds[ci]:bounds[ci + 1]] - ci * SH for ci in range(NCORES)]

    # L1 drops the appended self-loops (their q[d] is added on-device from
    # pos_shard); the E0 random edges keep identity node->row map (qtab).
    so1 = np.argsort(dst[:E0], kind="stable")
    dsts1 = dst[:E0][so1]
    srcs1 = src[:E0][so1]
    bounds1 = np.searchsorted(dsts1, np.arange(NCORES + 1) * SH)
    srcs1_pc = [srcs1[bounds1[ci]:bounds1[ci + 1]] for ci in range(NCORES)]
    dsts1_pc = [dsts1[bounds1[ci]:bounds1[ci + 1]] - ci * SH for ci in range(NCORES)]
    m1 = _edge_meta(cfg, srcs1_pc, dsts1_pc, lambda ci, s: s,
                    np.arange(NCH + 1) * CH)

    # L2: edges from MY OWN nodes read local gshard halves (rows [0, SH));
    # remote edges read the AllGathered halves at SH + core-major-half row
    # (row = h*8*HSH + core*HSH + r%HSH). Chunks: 2 local + 4 remote.
    def rowmap2(ci, s):
        c2 = s // SH
        r = s - c2 * SH
        h = r // HSH
        grow = h * (NCORES * HSH) + c2 * HSH + (r - h * HSH)
        return np.where(c2 == ci, r, SH + grow)
    m2 = _edge_meta(cfg, srcs_pc, dsts_pc, rowmap2,
                    [0, HSH, SH, SH + CH, SH + 2 * CH, SH + 3 * CH, SH + 4 * CH])

    # pooling metadata
    gbase = np.zeros(NCORES, np.int32)
    batch_local = np.full((NCORES, SH), 999.0, np.float32)
    for ci in range(NCORES):
        lo, hi = ci * SH, min((ci + 1) * SH, N)
        gbase[ci] = batch[lo]
        batch_local[ci, :hi - lo] = (batch[lo:hi] - batch[lo]).astype(np.float32)
        assert batch[hi - 1] - batch[lo] < P - 2, "too many graphs in one shard"
    cnts = np.bincount(batch, minlength=G).astype(np.float32)

    meta = dict(
        NPAD=NPAD, NT=NT, CH=CH, m1=m1, m2=m2, NPADROWS=NPAD - N,
    )
    # device-layout inputs (identical shapes across cores; values differ where noted)
    NTF = NPAD // P
    pos_pad = np.zeros((NPAD, c["IN_C"]), np.float32)
    pos_pad[:N] = pos
    pos_dev = pos_pad.reshape(NTF, P, c["IN_C"]).transpose(1, 0, 2).copy()
    pos_shard = [np.ascontiguousarray(
        pos_pad[ci * SH:(ci + 1) * SH].reshape(NT, P, c["IN_C"]).transpose(1, 0, 2))
        for ci in range(NCORES)]
    deg_dev = deg.reshape(NTF, P).T.copy()
    deg_shard = [deg[ci * SH:(ci + 1) * SH].reshape(NT, P).T.copy() for ci in range(NCORES)]
    batch_dev = [batch_local[ci].reshape(NT, P).T.copy() for ci in range(NCORES)]
    cnt_dev = np.zeros((P, (G + P - 1) // P), np.float32)
    for g in range(G):
        cnt_dev[g % P, g // P] = cnts[g]
    ins = []
    for ci in range(NCORES):
        ins.append(dict(
            pos_dev=pos_dev, deg_dev=deg_dev, deg_shard=deg_shard[ci],
            pos_shard=pos_shard[ci],
            idxs1=m1["idx"][ci], dstc1=m1["dstc"][ci],
            idxs2=m2["idx"][ci], dstc2=m2["dstc"][ci],
            batchl=batch_dev[ci], cntg=cnt_dev,
            gbase=np.array([[float(gbase[ci])]], np.float32),
        ))
    return meta, ins


def _prep_weights(cfg, W):
    """Re-layout weights for device (pure replication / transpose-free reshapes)."""
    c = cfg
    HID, OUT, IN_C, G = c["HID"], c["OUT"], c["IN_C"], c["G"]
    NH = HID // P                      # channel halves (2)
    w = {}
    w["w1"] = W["w_conv1"].astype(np.float16)                        # [2, 256]
    w["w1_cols"] = np.ascontiguousarray(
        np.asarray(W["w_conv1"], np.float32).T.reshape(NH, P, IN_C).transpose(1, 0, 2))  # [128, NH, 2]
    w["b1_cols"] = W["b_conv1"].reshape(NH, P).T.astype(np.float32).copy()   # [128, NH]
    w["ln1w_cols"] = W["ln1_w"].reshape(NH, P).T.astype(np.float32).copy()
    w["ln1b_cols"] = W["ln1_b"].reshape(NH, P).T.astype(np.float32).copy()
    w["w2_kt"] = np.ascontiguousarray(
        W["w_conv2"].reshape(NH, P, HID).transpose(1, 0, 2)).astype(np.float16)  # [128, NH, 256]
    w["b2_bc"] = np.tile(W["b_conv2"][None, :], (P, 1)).astype(np.float32)   # [128, 256]
    w["ln2w_bc"] = np.tile(W["ln2_w"][None, :], (P, 1)).astype(np.float32)
    w["ln2b_bc"] = np.tile(W["ln2_b"][None, :], (P, 1)).astype(np.float32)
    w["wl1_kt"] = np.ascontiguousarray(
        W["w_lin1"].reshape(NH, P, HID // 2).transpose(1, 0, 2)).astype(np.float16)  # [128, NH, 128]
    w["bl1_bc"] = np.tile(W["b_lin1"][None, :], (P, 1)).astype(np.float32)   # [128, 128]
    w["lnmw_bc"] = np.tile(W["lnm_w"][None, :], (P, 1)).astype(np.float32)
    w["lnmb_bc"] = np.tile(W["lnm_b"][None, :], (P, 1)).astype(np.float32)
    w["wl2"] = W["w_lin2"].astype(np.float32)                        # [128, 16]
    w["bl2_bc"] = np.tile(W["b_lin2"][None, :], (P, 1)).astype(np.float32)   # [128, 16]
    w["a1"] = float(W["a1"]); w["a2"] = float(W["a2"]); w["am"] = float(W["am"])
    return w


# ----------------------------------------------------------------- device build

def build_program(cfg, meta, weights):
    import concourse.bass as bass
    import concourse.mybir as mybir
    import concourse.tile as tile
    from concourse import bacc
    from concourse.masks import make_identity

    c = cfg
    dt = mybir.dt
    N, G, HID, OUT, IN_C = c["N"], c["G"], c["HID"], c["OUT"], c["IN_C"]
    SH, NCH = c["SH"], c["NCHUNK"]
    NCORES = c["NCORES"]
    NPAD, NT, CH = meta["NPAD"], meta["NT"], meta["CH"]
    em1, em2 = meta["m1"], meta["m2"]
    TOT1, TOT2 = em1["TOT"], em2["TOT"]
    HSH = SH // 2
    NTF = NPAD // P
    NH = HID // P
    GT = (G + P - 1) // P              # graph tiles (4)
    NPADROWS = meta["NPADROWS"]
    EPS = 1e-5
    CORE_IDS = list(range(NCORES))
    f32, f16, i16, i32 = dt.float32, dt.float16, dt.int16, dt.int32
    AF = mybir.ActivationFunctionType
    OP = mybir.AluOpType

    nc = bacc.Bacc("TRN2", debug=False, num_devices=NCORES, num_swdge_queues=4)

    # ---- I/O ----
    pos_in = nc.declare_dram_parameter("pos_dev", [P, NTF, IN_C], f32, isOutput=False)
    deg_in = nc.declare_dram_parameter("deg_dev", [P, NTF], f32, isOutput=False)
    degs_in = nc.declare_dram_parameter("deg_shard", [P, NT], f32, isOutput=False)
    poss_in = nc.declare_dram_parameter("pos_shard", [P, NT, IN_C], f32, isOutput=False)
    idx1_in = nc.declare_dram_parameter("idxs1", [P, TOT1 * 8], i16, isOutput=False)
    dstc1_in = nc.declare_dram_parameter("dstc1", [P, TOT1], f32, isOutput=False)
    idx2_in = nc.declare_dram_parameter("idxs2", [P, TOT2 * 8], i16, isOutput=False)
    dstc2_in = nc.declare_dram_parameter("dstc2", [P, TOT2], f32, isOutput=False)
    batch_in = nc.declare_dram_parameter("batchl", [P, NT], f32, isOutput=False)
    cnt_in = nc.declare_dram_parameter("cntg", [P, GT], f32, isOutput=False)
    gbase_in = nc.declare_dram_parameter("gbase", [1, 1], f32, isOutput=False)
    wt = {}
    wspec = dict(
        w1=([IN_C, HID], f16), w1_cols=([P, NH, IN_C], f32),
        b1_cols=([P, NH], f32), ln1w_cols=([P, NH], f32), ln1b_cols=([P, NH], f32),
        w2_kt=([P, NH, HID], f16), b2_bc=([P, HID], f32),
        ln2w_bc=([P, HID], f32), ln2b_bc=([P, HID], f32),
        wl1_kt=([P, NH, HID // 2], f16), bl1_bc=([P, HID // 2], f32),
        lnmw_bc=([P, HID // 2], f32), lnmb_bc=([P, HID // 2], f32),
        wl2=([HID // 2, OUT], f32), bl2_bc=([P, OUT], f32),
    )
    for k, (shp, dt_) in wspec.items():
        wt[k] = nc.declare_dram_parameter(k, shp, dt_, isOutput=False)
    out_ext = nc.declare_dram_parameter("out", [G, OUT], f32, isOutput=True)

    # ---- internal DRAM ----
    qtab = nc.dram_tensor("qtab", [NPAD, P], f16)                 # L1 table (2 real cols)
    # gshard/gtab split in two halves so the first AllGather overlaps the
    # second half's h2 compute and the L2 gathers of chunks 0-1.
    gshard0 = nc.dram_tensor("gshard0", [HSH, HID], f16)
    gshard1 = nc.dram_tensor("gshard1", [HSH, HID], f16)
    gtab0 = nc.dram_tensor("gtab0", [NCORES * HSH, HID], f16, addr_space="Shared")
    gtab1 = nc.dram_tensor("gtab1", [NCORES * HSH, HID], f16, addr_space="Shared")
    st1_in = nc.dram_tensor("st1_in", [1, P], f32)
    st1_out = nc.dram_tensor("st1_out", [1, P], f32, addr_space="Shared")
    st2_in = nc.dram_tensor("st2_in", [1, P], f32)
    st2_out = nc.dram_tensor("st2_out", [1, P], f32, addr_space="Shared")
    POOLR = GT * P                                                # 512 rows
    pool_in = nc.dram_tensor("pool_in", [POOLR, HID], f16)
    pool_out = nc.dram_tensor("pool_out", [POOLR, HID], f16, addr_space="Shared")

    a1, a2, am = weights["a1"], weights["a2"], weights["am"]

    with tile.TileContext(nc) as tc:
        with tc.tile_pool(name="persist", bufs=1) as pp, \
             tc.tile_pool(name="psc", bufs=4, space="PSUM") as psc:
            # ---- persistent small tiles ----
            iota_i = pp.tile([P, P], i32)
            nc.gpsimd.iota(iota_i[:], pattern=[[1, P]], base=0, channel_multiplier=0)
            iota_h = pp.tile([P, P], f16)
            nc.vector.tensor_copy(out=iota_h[:], in_=iota_i[:])
            iota_f = pp.tile([P, P], f32)
            nc.vector.tensor_copy(out=iota_f[:], in_=iota_i[:])
            ident = pp.tile([P, P], f32)
            make_identity(nc, ident[:])
            ones_col = pp.tile([P, 1], f32)
            nc.vector.memset(ones_col[:], 1.0)
            ones_row = pp.tile([1, P], f32)
            nc.vector.memset(ones_row[:], 1.0)

            # dinv (full + shard)
            deg_f = pp.tile([P, NTF], f32)
            nc.sync.dma_start(out=deg_f[:], in_=deg_in[:])
            nc.vector.tensor_scalar(out=deg_f[:], in0=deg_f[:], scalar1=1.0,
                                    scalar2=None, op0=OP.max)
            nc.scalar.sqrt(deg_f[:], deg_f[:])
            dinv_f = pp.tile([P, NTF], f32)
            nc.vector.reciprocal(dinv_f[:], deg_f[:])
            deg_s = pp.tile([P, NT], f32)
            nc.sync.dma_start(out=deg_s[:], in_=degs_in[:])
            nc.vector.tensor_scalar(out=deg_s[:], in0=deg_s[:], scalar1=1.0,
                                    scalar2=None, op0=OP.max)
            nc.scalar.sqrt(deg_s[:], deg_s[:])
            dinv_s = pp.tile([P, NT], f32)
            nc.vector.reciprocal(dinv_s[:], deg_s[:])

            # ---- build q table: q = pos * dinv (fp16 rows of qtab) ----
            with tc.tile_pool(name="p0", bufs=1) as p0:
                pos_sb = p0.tile([P, NTF, IN_C], f32)
                nc.sync.dma_start(out=pos_sb[:], in_=pos_in[:])
                q16 = p0.tile([P, NTF, IN_C], f16)
                for ch in range(IN_C):
                    nc.vector.tensor_tensor(out=q16[:, :, ch], in0=pos_sb[:, :, ch],
                                            in1=dinv_f[:], op=OP.mult)
                # write per chunk so chunk-0 gathers start before the rest
                qtab_v = qtab[:].rearrange("(a b) d -> b a d", b=P)
                CHT = CH // P
                for chk in range(NCH):
                    nc.sync.dma_start(
                        out=qtab_v[:, chk * CHT:(chk + 1) * CHT, 0:IN_C],
                        in_=q16[:, chk * CHT:(chk + 1) * CHT, :])

            dstc16_1 = pp.tile([P, TOT1], f16)
            dstc16_2 = pp.tile([P, TOT2], f16)
            with tc.tile_pool(name="dstld", bufs=2) as dsp:
                dstc_sb1 = dsp.tile([P, TOT1], f32, tag="d1")
                nc.sync.dma_start(out=dstc_sb1[:], in_=dstc1_in[:])
                nc.vector.tensor_copy(out=dstc16_1[:], in_=dstc_sb1[:])
                dstc_sb2 = dsp.tile([P, TOT2], f32, tag="d2")
                nc.sync.dma_start(out=dstc_sb2[:], in_=dstc2_in[:])
                nc.vector.tensor_copy(out=dstc16_2[:], in_=dstc_sb2[:])

            # zero the pool staging buffer early (independent of everything)
            zero_t = pp.tile([P, HID], f16)
            nc.vector.memset(zero_t[:], 0.0)
            for j in range(GT):
                nc.sync.dma_start(out=pool_in[j * P:(j + 1) * P, :], in_=zero_t[:])

            wsb = {}
            for k, (shp, dt_) in wspec.items():
                wsb[k] = pp.tile(shp, dt_, name=f"w_{k}")
                nc.sync.dma_start(out=wsb[k][:], in_=wt[k][:])

            # helper: cross-partition scalar sum -> [1,1] psum -> sbuf tile
            def part_sum(src_col, w_):
                ps = psc.tile([1, src_col.shape[1]], f32, space="PSUM", tag="psc_scratch")
                nc.tensor.matmul(out=ps[:], lhsT=ones_col[:], rhs=src_col[:],
                                 start=True, stop=True)
                dstt = w_.tile([1, src_col.shape[1]], f32, tag="psum_scalar")
                nc.vector.tensor_copy(out=dstt[:], in_=ps[:])
                return dstt

            def bcast_col(vals_row, w_):
                """vals_row [1, k] -> [128, k] replicated."""
                k = vals_row.shape[1]
                ps = psc.tile([P, k], f32, space="PSUM", tag="psc_scratch")
                nc.tensor.matmul(out=ps[:], lhsT=ones_row[:], rhs=vals_row[:],
                                 start=True, stop=True)
                o = w_.tile([P, k], f32, tag="bcast_col")
                nc.vector.tensor_copy(out=o[:], in_=ps[:])
                return o

            # ============ gather + segsum pass (shared for L1 / L2) ============
            def agg_pass(table_fn, elem, rhs_w, accum, m_, idx_in, dstc16,
                         stage_pool, idxp, selp, psp):
                """table_fn(chk) -> dram region [CH, elem] f16;
                accum [128, NT, rhs_w] zeroed; m_: edge-meta dict."""
                tiles, CALLS = m_["tiles"], m_["CALLS"]
                gt_tile = 0        # global tile index
                call_i = 0
                off_cols = 0       # idx column offset (8 per tile)
                for chk in range(tiles.shape[0]):
                    order = [t for t in range(NT) for _ in range(int(tiles[chk, t]))]
                    # call partitioning for this chunk
                    pos_in_chunk = 0
                    chunk_tiles = int(tiles[chk].sum())
                    while pos_in_chunk < chunk_tiles:
                        k = min(c["CALL_TILES"], chunk_tiles - pos_in_chunk)
                        idx_t = idxp.tile([P, c["CALL_TILES"] * 8], i16, tag="idx")
                        nc.sync.dma_start(
                            out=idx_t[:, 0:k * 8],
                            in_=idx_in[:, off_cols:off_cols + k * 8])
                        gbuf = stage_pool.tile([P, c["CALL_TILES"], elem], f16, tag="g")
                        nc.gpsimd.dma_gather(
                            out_ap=gbuf[:, 0:k, :],
                            in_ap=table_fn(chk),
                            idxs_ap=idx_t[:, 0:k * 8],
                            num_idxs=k * P, num_idxs_reg=k * P,
                            elem_size=elem, single_packet=False,
                            queue_num=call_i % c["QUEUES"])
                        # batched one-hot build for the whole call (one DVE op)
                        sel_call = selp.tile([P, c["CALL_TILES"], P], f16, tag="sel")
                        _i = iota_h[:]
                        _db = dstc16[:, gt_tile:gt_tile + k]
                        iota_bc = bass.AP(_i.tensor, _i.offset,
                                          [list(_i.ap[0]), [0, k], list(_i.ap[1])])
                        dst_bc = bass.AP(_db.tensor, _db.offset,
                                         [list(_db.ap[0]), list(_db.ap[1]), [0, P]])
                        nc.vector.tensor_tensor(out=sel_call[:, 0:k, :], in0=iota_bc,
                                                in1=dst_bc, op=OP.is_equal)
                        # consume the k tiles of this call
                        for j in range(k):
                            t = order[pos_in_chunk + j]
                            first = (order[pos_in_chunk + j - 1] != t) if (pos_in_chunk + j) > 0 else True
                            # new psum group when dtile changes (cells are contiguous)
                            if first:
                                ps = psp.tile([P, rhs_w], f32, space="PSUM", tag="pacc")
                            last = (pos_in_chunk + j == chunk_tiles - 1) or \
                                   (order[pos_in_chunk + j + 1] != t)
                            nc.tensor.matmul(
                                out=ps[:], lhsT=sel_call[:, j, :],
                                rhs=gbuf[:, j, 0:rhs_w],
                                start=first, stop=last)
                            if last:
                                if chk == 0:
                                    nc.vector.tensor_copy(out=accum[:, t, :],
                                                          in_=ps[:])
                                else:
                                    nc.vector.tensor_add(out=accum[:, t, :],
                                                         in0=accum[:, t, :], in1=ps[:])
                            gt_tile += 1
                        pos_in_chunk += k
                        off_cols += k * 8
                        call_i += 1

            # ---- W1/b1 replicated reductions (for moments-based ln1 stats) ----
            # w1aux cols: 0:sum_w0 1:sum_w1 2:sum_w0^2 3:sum_w1^2 4:sum_w0w1
            #             5:sum_b1w0 6:sum_b1w1 7:sum_b1 8:sum_b1^2
            w1aux = pp.tile([1, 10], f32)
            with tc.tile_pool(name="w1a", bufs=1) as wap:
                w0 = wsb["w1_cols"][:, :, 0]
                w1c = wsb["w1_cols"][:, :, 1]
                b1c = wsb["b1_cols"][:]
                wa = wap.tile([P, 9], f32)
                scr = wap.tile([P, NH], f32)

                def _red(dst, src):
                    nc.vector.tensor_reduce(out=dst, in_=src,
                                            axis=mybir.AxisListType.X, op=OP.add)
                _red(wa[:, 0:1], w0)
                _red(wa[:, 1:2], w1c)
                nc.vector.tensor_tensor(out=scr[:], in0=w0, in1=w0, op=OP.mult)
                _red(wa[:, 2:3], scr[:])
                nc.vector.tensor_tensor(out=scr[:], in0=w1c, in1=w1c, op=OP.mult)
                _red(wa[:, 3:4], scr[:])
                nc.vector.tensor_tensor(out=scr[:], in0=w0, in1=w1c, op=OP.mult)
                _red(wa[:, 4:5], scr[:])
                nc.vector.tensor_tensor(out=scr[:], in0=b1c, in1=w0, op=OP.mult)
                _red(wa[:, 5:6], scr[:])
                nc.vector.tensor_tensor(out=scr[:], in0=b1c, in1=w1c, op=OP.mult)
                _red(wa[:, 6:7], scr[:])
                _red(wa[:, 7:8], b1c)
                nc.vector.tensor_tensor(out=scr[:], in0=b1c, in1=b1c, op=OP.mult)
                _red(wa[:, 8:9], scr[:])
                t9 = part_sum(wa, wap)
                nc.vector.tensor_copy(out=w1aux[:, 0:9], in_=t9[:])

            # =============================== L1 ===============================
            with tc.tile_pool(name="acc1", bufs=1) as a1p:
                accum1 = a1p.tile([P, NT, IN_C], f32)
                with tc.tile_pool(name="st1", bufs=8) as sp1, \
                     tc.tile_pool(name="idx1", bufs=8) as ip1, \
                     tc.tile_pool(name="sel1", bufs=6) as sl1, \
                     tc.tile_pool(name="ps1", bufs=3, space="PSUM") as pp1, \
                     nc.named_scope("L1agg"):
                    agg_pass(lambda chk: qtab[chk * CH:(chk + 1) * CH, :],
                             P, IN_C, accum1, em1, idx1_in, dstc16_1,
                             sp1, ip1, sl1, pp1)

                # ---- ln1 stats from moments of s = dinv*agg (no x1 pass) ----
                with tc.tile_pool(name="x1", bufs=1) as x1p, nc.named_scope("mid"):
                    x1t = x1p.tile([P, NH, SH], f16)
                    acol = x1p.tile([P, NH], f32)
                    bcol = x1p.tile([P, NH], f32)
                    _dv = dinv_s[:]
                    dinv_bc2 = bass.AP(_dv.tensor, _dv.offset,
                                       [list(_dv.ap[0]), list(_dv.ap[1]), [0, IN_C]])
                    # add the self-loop term q[d] = pos[d]*dinv[d] locally
                    # (L1 edge streams exclude the appended self-loops)
                    poss_sb = x1p.tile([P, NT, IN_C], f32)
                    nc.sync.dma_start(out=poss_sb[:], in_=poss_in[:])
                    selfq = x1p.tile([P, NT, IN_C], f32)
                    nc.vector.tensor_tensor(out=selfq[:], in0=poss_sb[:],
                                            in1=dinv_bc2, op=OP.mult)
                    nc.vector.tensor_add(out=accum1[:], in0=accum1[:], in1=selfq[:])
                    sct = x1p.tile([P, NT, IN_C], f16)
                    nc.vector.tensor_tensor(out=sct[:], in0=accum1[:], in1=dinv_bc2,
                                            op=OP.mult)
                    with tc.tile_pool(name="st1p", bufs=1) as stp:
                        pr = stp.tile([P, NT], f16)
                        cols = stp.tile([P, 5], f32)
                        nc.vector.tensor_reduce(out=cols[:, 0:1], in_=sct[:, :, 0],
                                                axis=mybir.AxisListType.X, op=OP.add)
                        nc.vector.tensor_reduce(out=cols[:, 1:2], in_=sct[:, :, 1],
                                                axis=mybir.AxisListType.X, op=OP.add)
                        nc.vector.tensor_tensor(out=pr[:], in0=sct[:, :, 0],
                                                in1=sct[:, :, 0], op=OP.mult)
                        nc.vector.tensor_reduce(out=cols[:, 2:3], in_=pr[:],
                                                axis=mybir.AxisListType.X, op=OP.add)
                        nc.vector.tensor_tensor(out=pr[:], in0=sct[:, :, 1],
                                                in1=sct[:, :, 1], op=OP.mult)
                        nc.vector.tensor_reduce(out=cols[:, 3:4], in_=pr[:],
                                                axis=mybir.AxisListType.X, op=OP.add)
                        nc.vector.tensor_tensor(out=pr[:], in0=sct[:, :, 0],
                                                in1=sct[:, :, 1], op=OP.mult)
                        nc.vector.tensor_reduce(out=cols[:, 4:5], in_=pr[:],
                                                axis=mybir.AxisListType.X, op=OP.add)
                        tot = part_sum(cols, stp)            # [1,5] local S0,S1,M00,M11,M01
                        arr = stp.tile([1, P], f32)
                        nc.vector.memset(arr[:], 0.0)
                        nc.vector.tensor_copy(out=arr[:, 0:5], in_=tot[:])
                        nc.sync.dma_start(out=st1_in[:], in_=arr[:])
                        nc.gpsimd.collective_compute(
                            "AllReduce", OP.add, replica_groups=[CORE_IDS],
                            ins=[st1_in[:]], outs=[st1_out[:]])
                        arro = stp.tile([1, P], f32)
                        nc.sync.dma_start(out=arro[:], in_=st1_out[:])
                        # sumx = S.w1sum + N*b1sum ; sumsq = quad(M,w1) + 2*S.b1w + N*b1sq
                        CNT = float(N * HID)
                        s2 = stp.tile([1, 2], f32)
                        nc.vector.tensor_tensor(out=s2[:], in0=arro[:, 0:2],
                                                in1=w1aux[:, 0:2], op=OP.mult)
                        sumx = stp.tile([1, 1], f32)
                        nc.vector.tensor_add(out=sumx[:], in0=s2[:, 0:1], in1=s2[:, 1:2])
                        t1 = stp.tile([1, 1], f32)
                        nc.vector.tensor_scalar(out=t1[:], in0=w1aux[:, 7:8],
                                                scalar1=float(N), scalar2=None, op0=OP.mult)
                        nc.vector.tensor_add(out=sumx[:], in0=sumx[:], in1=t1[:])
                        q3 = stp.tile([1, 3], f32)
                        nc.vector.tensor_tensor(out=q3[:], in0=arro[:, 2:5],
                                                in1=w1aux[:, 2:5], op=OP.mult)
                        sb2 = stp.tile([1, 2], f32)
                        nc.vector.tensor_tensor(out=sb2[:], in0=arro[:, 0:2],
                                                in1=w1aux[:, 5:7], op=OP.mult)
                        sumsq = stp.tile([1, 1], f32)
                        nc.vector.tensor_add(out=sumsq[:], in0=q3[:, 0:1], in1=q3[:, 1:2])
                        nc.vector.tensor_scalar(out=t1[:], in0=q3[:, 2:3],
                                                scalar1=2.0, scalar2=None, op0=OP.mult)
                        nc.vector.tensor_add(out=sumsq[:], in0=sumsq[:], in1=t1[:])
                        nc.vector.tensor_add(out=t1[:], in0=sb2[:, 0:1], in1=sb2[:, 1:2])
                        nc.vector.tensor_scalar(out=t1[:], in0=t1[:],
                                                scalar1=2.0, scalar2=None, op0=OP.mult)
                        nc.vector.tensor_add(out=sumsq[:], in0=sumsq[:], in1=t1[:])
                        nc.vector.tensor_scalar(out=t1[:], in0=w1aux[:, 8:9],
                                                scalar1=float(N), scalar2=None, op0=OP.mult)
                        nc.vector.tensor_add(out=sumsq[:], in0=sumsq[:], in1=t1[:])
                        mean_t = stp.tile([1, 1], f32)
                        nc.vector.tensor_scalar(out=mean_t[:], in0=sumx[:],
                                                scalar1=1.0 / CNT, scalar2=None, op0=OP.mult)
                        ex2 = stp.tile([1, 1], f32)
                        nc.vector.tensor_scalar(out=ex2[:], in0=sumsq[:],
                                                scalar1=1.0 / CNT, scalar2=None, op0=OP.mult)
                        m2 = stp.tile([1, 1], f32)
                        nc.vector.tensor_tensor(out=m2[:], in0=mean_t[:], in1=mean_t[:],
                                                op=OP.mult)
                        var = stp.tile([1, 1], f32)
                        nc.vector.tensor_tensor(out=var[:], in0=ex2[:], in1=m2[:],
                                                op=OP.subtract)
                        nc.scalar.sqrt(var[:], var[:])
                        nc.vector.tensor_scalar(out=var[:], in0=var[:], scalar1=EPS,
                                                scalar2=None, op0=OP.add)
                        rstd = stp.tile([1, 1], f32)
                        nc.vector.reciprocal(rstd[:], var[:])
                        pack = stp.tile([1, 2], f32)
                        nc.vector.tensor_copy(out=pack[:, 0:1], in_=mean_t[:])
                        nc.vector.tensor_copy(out=pack[:, 1:2], in_=rstd[:])
                        mr = bcast_col(pack, stp)            # [128,2] (mean, rstd)
                        # x1' = (x1-mu)*rstd*w + b = x1*acol + bcol (per-channel)
                        nc.vector.tensor_scalar(out=acol[:], in0=wsb["ln1w_cols"][:],
                                                scalar1=mr[:, 1:2], scalar2=None, op0=OP.mult)
                        nc.vector.tensor_scalar(out=bcol[:], in0=wsb["b1_cols"][:],
                                                scalar1=mr[:, 0:1], scalar2=None,
                                                op0=OP.subtract)
                        nc.vector.tensor_tensor(out=bcol[:], in0=bcol[:], in1=acol[:],
                                                op=OP.mult)
                        nc.vector.tensor_add(out=bcol[:], in0=bcol[:],
                                             in1=wsb["ln1b_cols"][:])

                    # ---- fused per tile: x1 = prelu(a*(W1@s.T)+b) then
                    #      h2 = x1' @ W2; g = dinv*h2 -> gshard halves ----
                    with tc.tile_pool(name="w2w", bufs=6) as wk, \
                         tc.tile_pool(name="px1", bufs=2, space="PSUM") as px1, \
                         tc.tile_pool(name="h2w", bufs=4) as h2w, \
                         tc.tile_pool(name="ph2", bufs=2, space="PSUM") as ph2:
                        for t in range(NT):
                            sc = wk.tile([P, IN_C], f32, tag="sc")
                            nc.vector.tensor_scalar(
                                out=sc[:], in0=accum1[:, t, :],
                                scalar1=dinv_s[:, t:t + 1], scalar2=None, op0=OP.mult)
                            pt = psc.tile([IN_C, P], f32, space="PSUM", tag="psc_scratch")
                            nc.tensor.transpose(out=pt[:], in_=sc[:], identity=ident[:])
                            p1t = wk.tile([IN_C, P], f16, tag="p1t_sb")
                            nc.vector.tensor_copy(out=p1t[:], in_=pt[:])
                            for h in range(NH):
                                psx = px1.tile([P, P], f32, space="PSUM", tag="px1")
                                nc.tensor.matmul(
                                    out=psx[:], lhsT=wsb["w1"][:, h * P:(h + 1) * P],
                                    rhs=p1t[:], start=True, stop=True)
                                nc.scalar.activation(
                                    out=x1t[:, h, t * P:(t + 1) * P], in_=psx[:],
                                    func=AF.Prelu, bias=bcol[:, h:h + 1],
                                    scale=acol[:, h:h + 1], alpha=a1)
                            ps2 = ph2.tile([P, HID], f32, space="PSUM", tag="ph2")
                            for h in range(NH):
                                nc.tensor.matmul(
                                    out=ps2[:], lhsT=x1t[:, h, t * P:(t + 1) * P],
                                    rhs=wsb["w2_kt"][:, h, :], start=(h == 0), stop=(h == NH - 1))
                            g16 = h2w.tile([P, HID], f16, tag="g16")
                            nc.vector.tensor_scalar(
                                out=g16[:], in0=ps2[:],
                                scalar1=dinv_s[:, t:t + 1], scalar2=None, op0=OP.mult)
                            if t < NT // 2:
                                nc.sync.dma_start(
                                    out=gshard0[t * P:(t + 1) * P, :], in_=g16[:])
                            else:
                                t_ = t - NT // 2
                                nc.sync.dma_start(
                                    out=gshard1[t_ * P:(t_ + 1) * P, :], in_=g16[:])
            # pools a1p/x1p closed
            with nc.named_scope("allgather"):
                nc.gpsimd.collective_compute(
                    "AllGather", OP.bypass, replica_groups=[CORE_IDS],
                    ins=[gshard0[:]], outs=[gtab0[:]])
                nc.gpsimd.collective_compute(
                    "AllGather", OP.bypass, replica_groups=[CORE_IDS],
                    ins=[gshard1[:]], outs=[gtab1[:]])

            # =============================== L2 ===============================
            with tc.tile_pool(name="acc2", bufs=1) as a2p:
                accum2 = a2p.tile([P, NT, HID], f16)

                def l2_table(chk):
                    if chk == 0:
                        return gshard0[:]
                    if chk == 1:
                        return gshard1[:]
                    gt_ = gtab0 if chk < 4 else gtab1
                    base = (chk - 2) % 2
                    return gt_[base * CH:(base + 1) * CH, :]
                with tc.tile_pool(name="st2", bufs=5) as sp2, \
                     tc.tile_pool(name="idx2", bufs=8) as ip2, \
                     tc.tile_pool(name="sel2", bufs=4) as sl2, \
                     tc.tile_pool(name="ps2", bufs=4, space="PSUM") as pp2, \
                     nc.named_scope("L2agg"):
                    agg_pass(l2_table, HID, HID, accum2, em2, idx2_in, dstc16_2,
                             sp2, ip2, sl2, pp2)

                # ---- x2 = dinv*agg + b2 per-tile (pipelines into L2agg tail),
                #      ln2 stats, prelu ----
                with tc.tile_pool(name="stats2", bufs=1) as stp:
                    b2_16 = stp.tile([P, HID], f16)
                    nc.vector.tensor_copy(out=b2_16[:], in_=wsb["b2_bc"][:])
                    s_cols2 = stp.tile([P, NT], f32)
                    sq_cols2 = stp.tile([P, NT], f32)
                    with tc.tile_pool(name="sqscr", bufs=4) as sqp:
                        for t in range(NT):
                            nc.vector.tensor_scalar(
                                out=accum2[:, t, :], in0=accum2[:, t, :],
                                scalar1=dinv_s[:, t:t + 1], scalar2=None,
                                op0=OP.mult)
                            nc.vector.tensor_tensor(out=accum2[:, t, :],
                                                    in0=accum2[:, t, :],
                                                    in1=b2_16[:], op=OP.add)
                            sq_scr2 = sqp.tile([P, HID], f16, tag="sqs")
                            nc.scalar.activation(out=sq_scr2[:], in_=accum2[:, t, :],
                                                 func=AF.Square,
                                                 accum_out=sq_cols2[:, t:t + 1])
                            nc.vector.tensor_reduce(out=s_cols2[:, t:t + 1],
                                                    in_=accum2[:, t, :],
                                                    axis=mybir.AxisListType.X,
                                                    op=OP.add)
                    s_col = stp.tile([P, 1], f32)
                    nc.vector.tensor_reduce(out=s_col[:], in_=s_cols2[:],
                                            axis=mybir.AxisListType.X, op=OP.add)
                    sq_col = stp.tile([P, 1], f32)
                    nc.vector.tensor_reduce(out=sq_col[:], in_=sq_cols2[:],
                                            axis=mybir.AxisListType.X, op=OP.add)
                    both = stp.tile([P, 2], f32)
                    nc.vector.tensor_copy(out=both[:, 0:1], in_=s_col[:])
                    nc.vector.tensor_copy(out=both[:, 1:2], in_=sq_col[:])
                    tot = part_sum(both, stp)
                    # b2 pad corrections (pad rows equal b2 exactly)
                    b2p = stp.tile([1, 2], f32)
                    nc.vector.tensor_reduce(out=b2p[:, 0:1], in_=wsb["b2_bc"][0:1, :],
                                            axis=mybir.AxisListType.X, op=OP.add)
                    b2sq = stp.tile([1, HID], f32)
                    nc.scalar.square(b2sq[:], wsb["b2_bc"][0:1, :])
                    nc.vector.tensor_reduce(out=b2p[:, 1:2], in_=b2sq[:],
                                            axis=mybir.AxisListType.X, op=OP.add)
                    arr = stp.tile([1, P], f32)
                    nc.vector.memset(arr[:], 0.0)
                    nc.vector.tensor_copy(out=arr[:, 0:2], in_=tot[:])
                    nc.sync.dma_start(out=st2_in[:], in_=arr[:])
                    nc.gpsimd.collective_compute(
                        "AllReduce", OP.add, replica_groups=[CORE_IDS],
                        ins=[st2_in[:]], outs=[st2_out[:]])
                    arro = stp.tile([1, P], f32)
                    nc.sync.dma_start(out=arro[:], in_=st2_out[:])
                    CNT = float(N * HID)
                    cor = stp.tile([1, 2], f32)
                    nc.vector.tensor_scalar(out=cor[:], in0=b2p[:],
                                            scalar1=-float(NPADROWS), scalar2=None, op0=OP.mult)
                    nc.vector.tensor_add(out=cor[:], in0=cor[:], in1=arro[:, 0:2])
                    mean_t = stp.tile([1, 1], f32)
                    nc.vector.tensor_scalar(out=mean_t[:], in0=cor[:, 0:1],
                                            scalar1=1.0 / CNT, scalar2=None, op0=OP.mult)
                    ex2 = stp.tile([1, 1], f32)
                    nc.vector.tensor_scalar(out=ex2[:], in0=cor[:, 1:2],
                                            scalar1=1.0 / CNT, scalar2=None, op0=OP.mult)
                    m2 = stp.tile([1, 1], f32)
                    nc.vector.tensor_tensor(out=m2[:], in0=mean_t[:], in1=mean_t[:], op=OP.mult)
                    var = stp.tile([1, 1], f32)
                    nc.vector.tensor_tensor(out=var[:], in0=ex2[:], in1=m2[:], op=OP.subtract)
                    nc.scalar.sqrt(var[:], var[:])
                    nc.vector.tensor_scalar(out=var[:], in0=var[:], scalar1=EPS,
                                            scalar2=None, op0=OP.add)
                    rstd = stp.tile([1, 1], f32)
                    nc.vector.reciprocal(rstd[:], var[:])
                    pack = stp.tile([1, 2], f32)
                    nc.vector.tensor_copy(out=pack[:, 0:1], in_=mean_t[:])
                    nc.vector.tensor_copy(out=pack[:, 1:2], in_=rstd[:])
                    mr = bcast_col(pack, stp)
                    a_bc = stp.tile([P, HID], f32)
                    c_bc = stp.tile([P, HID], f32)
                    nc.vector.tensor_scalar(out=a_bc[:], in0=wsb["ln2w_bc"][:],
                                            scalar1=mr[:, 1:2], scalar2=None, op0=OP.mult)
                    nc.vector.tensor_scalar(out=c_bc[:], in0=a_bc[:],
                                            scalar1=mr[:, 0:1], scalar2=None, op0=OP.mult)
                    nc.vector.tensor_tensor(out=c_bc[:], in0=wsb["ln2b_bc"][:],
                                            in1=c_bc[:], op=OP.subtract)
                    a_16 = a2p.tile([P, HID], f16)
                    c_16 = a2p.tile([P, HID], f16)
                    nc.vector.tensor_copy(out=a_16[:], in_=a_bc[:])
                    nc.vector.tensor_copy(out=c_16[:], in_=c_bc[:])

                # =========================== pooling ===========================
                with tc.tile_pool(name="poolp", bufs=1) as plp, \
                     tc.tile_pool(name="pps", bufs=1, space="PSUM") as pps, \
                     nc.named_scope("tail"):
                    batch_sb = plp.tile([P, NT], f32)
                    nc.sync.dma_start(out=batch_sb[:], in_=batch_in[:])
                    psg = pps.tile([P, HID], f32, space="PSUM", tag="psg")
                    with tc.tile_pool(name="selg", bufs=6) as slg:
                        for t in range(NT):
                            nc.vector.tensor_tensor(out=accum2[:, t, :],
                                                    in0=accum2[:, t, :],
                                                    in1=a_16[:], op=OP.mult)
                            nc.vector.tensor_tensor(out=accum2[:, t, :],
                                                    in0=accum2[:, t, :],
                                                    in1=c_16[:], op=OP.add)
                            nc.scalar.activation(out=accum2[:, t, :],
                                                 in_=accum2[:, t, :],
                                                 func=AF.Prelu, alpha=a2)
                            selg = slg.tile([P, P], f16, tag="selg")
                            nc.vector.tensor_scalar(
                                out=selg[:], in0=iota_h[:],
                                scalar1=batch_sb[:, t:t + 1], scalar2=None,
                                op0=OP.is_equal)
                            nc.tensor.matmul(out=psg[:], lhsT=selg[:],
                                             rhs=accum2[:, t, :],
                                             start=(t == 0), stop=(t == NT - 1))
                    partial = plp.tile([P, HID], f16)
                    nc.vector.tensor_copy(out=partial[:], in_=psg[:])
                    # place rows at graph_base via one-hot matmuls; zero the rest
                    gb_sb = plp.tile([1, 1], f32)
                    nc.sync.dma_start(out=gb_sb[:], in_=gbase_in[:])
                    gb_col = bcast_col(gb_sb, plp)           # [128,1]
                    loc_col = plp.tile([P, 1], f32)
                    nc.vector.tensor_copy(out=loc_col[:], in_=iota_f[:, 0:1])
                    # iota column: iota_f[:,0:1] is all zeros (values are along free dim)
                    # need per-partition index 0..127: use iota with channel_multiplier=1
                    pidx_i = plp.tile([P, 1], i32)
                    nc.gpsimd.iota(pidx_i[:], pattern=[[0, 1]], base=0, channel_multiplier=1)
                    pidx = plp.tile([P, 1], f32)
                    nc.vector.tensor_copy(out=pidx[:], in_=pidx_i[:])
                    nc.vector.tensor_add(out=loc_col[:], in0=pidx[:], in1=gb_col[:])
                    with tc.tile_pool(name="plc", bufs=3) as plc, \
                         tc.tile_pool(name="ppl", bufs=2, space="PSUM") as ppl:
                        for j in range(GT):
                            sh_col = plc.tile([P, 1], f32, tag="shc")
                            nc.vector.tensor_scalar(out=sh_col[:], in0=loc_col[:],
                                                    scalar1=-float(j * P), scalar2=None,
                                                    op0=OP.add)
                            selj = plc.tile([P, P], f16, tag="selj")
                            nc.vector.tensor_scalar(out=selj[:], in0=iota_h[:],
                                                    scalar1=sh_col[:], scalar2=None,
                                                    op0=OP.is_equal)
                            psj = ppl.tile([P, HID], f32, space="PSUM", tag="psj")
                            nc.tensor.matmul(out=psj[:], lhsT=selj[:], rhs=partial[:],
                                             start=True, stop=True)
                            oj = plc.tile([P, HID], f16, tag="oj")
                            nc.vector.tensor_copy(out=oj[:], in_=psj[:])
                            nc.sync.dma_start(out=pool_in[j * P:(j + 1) * P, :], in_=oj[:])
                    nc.gpsimd.collective_compute(
                        "AllReduce", OP.add, replica_groups=[CORE_IDS],
                        ins=[pool_in[:]], outs=[pool_out[:]])

                    # ---- head (redundant on every core) ----
                    cnt_sb = plp.tile([P, GT], f32)
                    nc.sync.dma_start(out=cnt_sb[:], in_=cnt_in[:])
                    nc.vector.tensor_scalar(out=cnt_sb[:], in0=cnt_sb[:], scalar1=1.0,
                                            scalar2=None, op0=OP.max)
                    rec_sb = plp.tile([P, GT], f32)
                    nc.vector.reciprocal(rec_sb[:], cnt_sb[:])
                    pooled16r = plp.tile([P, GT, HID], f16)
                    nc.sync.dma_start(
                        out=pooled16r[:],
                        in_=pool_out[0:G, :].rearrange("(a b) d -> b a d", b=P))
                    pooled = plp.tile([P, GT, HID], f32)
                    for j in range(GT):
                        nc.vector.tensor_scalar(out=pooled[:, j, :],
                                                in0=pooled16r[:, j, :],
                                                scalar1=rec_sb[:, j:j + 1], scalar2=None,
                                                op0=OP.mult)
                    # pooledT [128ch, NH, G] (f16 for the head matmuls)
                    pooledT = plp.tile([P, NH, G], f16)
                    with tc.tile_pool(name="trp", bufs=2) as trp:
                        for j in range(GT):
                            for h in range(NH):
                                ptp = psc.tile([P, P], f32, space="PSUM", tag="psc_scratch")
                                nc.tensor.transpose(
                                    out=ptp[:], in_=pooled[:, j, h * P:(h + 1) * P],
                                    identity=ident[:])
                                nc.vector.tensor_copy(
                                    out=pooledT[:, h, j * P:(j + 1) * P], in_=ptp[:])
                    # h1 = pooled @ Wl1 + bl1 : [G, 128]
                    HW = HID // 2
                    h1 = plp.tile([P, GT, HW], f32)
                    with tc.tile_pool(name="ph1", bufs=2, space="PSUM") as ph1:
                        for j in range(GT):
                            psh = ph1.tile([P, HW], f32, space="PSUM", tag="psh")
                            for h in range(NH):
                                nc.tensor.matmul(
                                    out=psh[:], lhsT=pooledT[:, h, j * P:(j + 1) * P],
                                    rhs=wsb["wl1_kt"][:, h, :], start=(h == 0), stop=(h == NH - 1))
                            nc.vector.tensor_add(out=h1[:, j, :], in0=psh[:],
                                                 in1=wsb["bl1_bc"][:])
                    # lnm (local, exact: G*HW elements)
                    s_col = plp.tile([P, 1], f32)
                    nc.vector.tensor_reduce(out=s_col[:], in_=h1[:].rearrange("p a b -> p (a b)"),
                                            axis=mybir.AxisListType.X, op=OP.add)
                    sq_col = plp.tile([P, 1], f32)
                    sqt2 = plp.tile([P, GT * HW], f32)
                    nc.scalar.activation(out=sqt2[:], in_=h1[:].rearrange("p a b -> p (a b)"),
                                         func=AF.Square, accum_out=sq_col[:])
                    both = plp.tile([P, 2], f32)
                    nc.vector.tensor_copy(out=both[:, 0:1], in_=s_col[:])
                    nc.vector.tensor_copy(out=both[:, 1:2], in_=sq_col[:])
                    tot = part_sum(both, plp)
                    CNTM = float(G * HW)
                    mean_t = plp.tile([1, 1], f32)
                    nc.vector.tensor_scalar(out=mean_t[:], in0=tot[:, 0:1],
                                            scalar1=1.0 / CNTM, scalar2=None, op0=OP.mult)
                    ex2 = plp.tile([1, 1], f32)
                    nc.vector.tensor_scalar(out=ex2[:], in0=tot[:, 1:2],
                                            scalar1=1.0 / CNTM, scalar2=None, op0=OP.mult)
                    m2 = plp.tile([1, 1], f32)
                    nc.vector.tensor_tensor(out=m2[:], in0=mean_t[:], in1=mean_t[:], op=OP.mult)
                    var = plp.tile([1, 1], f32)
                    nc.vector.tensor_tensor(out=var[:], in0=ex2[:], in1=m2[:], op=OP.subtract)
                    nc.scalar.sqrt(var[:], var[:])
                    nc.vector.tensor_scalar(out=var[:], in0=var[:], scalar1=EPS,
                                            scalar2=None, op0=OP.add)
                    rstd = plp.tile([1, 1], f32)
                    nc.vector.reciprocal(rstd[:], var[:])
                    pack = plp.tile([1, 2], f32)
                    nc.vector.tensor_copy(out=pack[:, 0:1], in_=mean_t[:])
                    nc.vector.tensor_copy(out=pack[:, 1:2], in_=rstd[:])
                    mr = bcast_col(pack, plp)
                    a_bc = plp.tile([P, HW], f32)
                    c_bc = plp.tile([P, HW], f32)
                    nc.vector.tensor_scalar(out=a_bc[:], in0=wsb["lnmw_bc"][:],
                                            scalar1=mr[:, 1:2], scalar2=None, op0=OP.mult)
                    nc.vector.tensor_scalar(out=c_bc[:], in0=a_bc[:],
                                            scalar1=mr[:, 0:1], scalar2=None, op0=OP.mult)
                    nc.vector.tensor_tensor(out=c_bc[:], in0=wsb["lnmb_bc"][:],
                                            in1=c_bc[:], op=OP.subtract)
                    for j in range(GT):
                        nc.vector.tensor_tensor(out=h1[:, j, :], in0=h1[:, j, :],
                                                in1=a_bc[:], op=OP.mult)
                        nc.vector.tensor_add(out=h1[:, j, :], in0=h1[:, j, :], in1=c_bc[:])
                    nc.scalar.activation(out=h1[:].rearrange("p a b -> p (a b)"),
                                         in_=h1[:].rearrange("p a b -> p (a b)"),
                                         func=AF.Prelu, alpha=am)
                    # out = h1' @ wl2 + bl2
                    outt = plp.tile([P, GT, OUT], f32)
                    with tc.tile_pool(name="of", bufs=2) as ofp:
                        for j in range(GT):
                            ptp = psc.tile([P, P], f32, space="PSUM", tag="psc_scratch")
                            nc.tensor.transpose(out=ptp[:], in_=h1[:, j, :],
                                                identity=ident[:])
                            h1t = ofp.tile([P, P], f32, tag="h1t")
                            nc.vector.tensor_copy(out=h1t[:], in_=ptp[:])
                            pso = psc.tile([P, OUT], f32, space="PSUM", tag="psc_scratch")
                            nc.tensor.matmul(out=pso[:], lhsT=h1t[:], rhs=wsb["wl2"][:],
                                             start=True, stop=True)
                            nc.vector.tensor_add(out=outt[:, j, :], in0=pso[:],
                                                 in1=wsb["bl2_bc"][:, 0:OUT])
                    nc.sync.dma_start(
                        out=out_ext[:].rearrange("(a b) d -> b a d", b=P),
                        in_=outt[:])

    nc.compile()
    return nc


# ----------------------------------------------------------------- entry point

def _run(cfg, inputs, use_sim=False):
    import sys
    if '/opt/trn_rl_repo' not in sys.path:
        sys.path.insert(0, '/opt/trn_rl_repo')
    pos = np.asarray(inputs["pos"], np.float32)
    ei = np.asarray(inputs["edge_index"], np.int64)
    batch = np.asarray(inputs["batch"], np.int64)
    meta, core_ins = host_prep(cfg, pos, ei, batch)
    w = _prep_weights(cfg, inputs)
    nc = build_program(cfg, meta, w)
    for ci in range(cfg["NCORES"]):
        for k, v in w.items():
            if isinstance(v, np.ndarray):
                core_ins[ci][k] = v
    if use_sim:
        from concourse.bass_interp import MultiCoreSim
        sim = MultiCoreSim(nc, cfg["NCORES"], require_finite=False,
                           require_nnan=False)
        for ci in range(cfg["NCORES"]):
            for k, v in core_ins[ci].items():
                sim.cores[ci].tensor(k)[:] = v
        sim.simulate()
        return np.array(sim.cores[0].tensor("out")), None
    from concourse.bass_utils import run_bass_kernel_spmd
    res = run_bass_kernel_spmd(nc, core_ins, list(range(cfg["NCORES"])))
    return res.results[0]["out"], res


def kernel(**inputs):
    out, _ = _run(_cfg_full(), inputs)
    return out



# revision 65
# speedup vs baseline: 1.5703x; 1.5703x over previous
"""GCN (2x GCNConv + graph-layernorm + prelu + mean-pool + MLP head) on 8 trn2 cores.

Strategy (dst-sharded graph parallel):
  - nodes (and their incoming edges) sharded 8 ways by dst; weights replicated.
  - per-edge gather of source features via gpsimd dma_gather (fp16 table rows),
    segment-sum via one-hot selection matmuls on the TensorEngine (PSUM
    accumulate, fp16 accumulators in SBUF across src chunks).
  - L1 aggregates the 2-channel scaled positions; its graph-layernorm stats
    come from closed-form moments of s = dinv*agg (sum + 2x2 second moment),
    so no stats pass over x1 is needed; affine+prelu fold into per-tile
    Activation ops right out of PSUM (x1 kept fp16, transposed layout).
  - h2 = x1' @ W2 tables are AllGathered in fp16 as TWO half-shard
    collectives so the first overlaps the rest of h2 and the L2 gathers of
    its half; edges to a core's own nodes gather the local gshard halves
    without waiting for any collective (6-chunk layout, host-remapped rows).
  - the x2 epilogue (dinv scale, +b2, square/sum stats) runs per dst tile so
    it pipelines into the L2 aggregation tail; ln2 via AllReduce of scalars.
  - per-graph mean-pool partials combined via fp16 AllReduce; MLP head
    computed redundantly on every core.
All floating point compute happens on device; the host only shards/sorts/pads
integer index metadata and re-lays-out inputs.
"""

import numpy as np

P = 128


def _cfg_tiny():
    return dict(
        N=1900, E0=8000, G=128, IN_C=2, HID=256, OUT=16,
        NCORES=8, SH=256, CH=512, NCHUNK=4, CALL_TILES=4, QUEUES=1,
    )


def _cfg_full():
    return dict(
        N=100000, E0=3200000, G=512, IN_C=2, HID=256, OUT=16,
        NCORES=8, SH=12544, CH=32768, NCHUNK=4, CALL_TILES=32, QUEUES=4,
    )


# ----------------------------------------------------------------- host prep

def _wrap_idx(ix):
    """dma_gather idx layout: idx i -> [i%16 + 16k, i//16] for all k (replicated)."""
    m = ix.reshape(-1, 16).T
    return np.tile(m, (8, 1)).astype(np.int16)


def _edge_meta(cfg, srcs_pc, dsts_pc, rowmap, bounds):
    """Bucket per-core edges into (chunk, dst-tile) cells under a src->row map.

    bounds: chunk boundaries in the (possibly per-core) row space.
    Returns dict with tiles/CALLS (SPMD-uniform) and per-core wrapped idx +
    dst one-hot column streams."""
    c = cfg
    NCORES, SH = c["NCORES"], c["SH"]
    NT = SH // P
    bounds = np.asarray(bounds, np.int64)
    NCH = len(bounds) - 1
    percore = []
    counts = np.zeros((NCORES, NCH, NT), np.int64)
    for ci in range(NCORES):
        row = rowmap(ci, srcs_pc[ci])
        d = dsts_pc[ci]
        chunk = np.searchsorted(bounds, row, side="right") - 1
        row = row - bounds[chunk]
        o = np.lexsort((d, chunk))
        row, d, chunk = row[o], d[o], chunk[o]
        cnt = np.bincount(chunk * NT + d // P, minlength=NCH * NT).reshape(NCH, NT)
        counts[ci] = cnt
        percore.append((row, d, cnt))

    tiles = np.maximum(1, (counts.max(axis=0) + P - 1) // P)   # [NCH, NT]
    tiles_chunk = tiles.sum(axis=1)                            # [NCH]
    TOT = int(tiles.sum())

    idx_streams, dst_streams = [], []
    for ci in range(NCORES):
        row, d, cnt = percore[ci]
        idxs = np.zeros(TOT * P, np.int16)
        dcol = np.full(TOT * P, 999.0, np.float32)
        off = 0
        eoff = np.concatenate([[0], np.cumsum(cnt.ravel())])
        for ch in range(NCH):
            for t in range(NT):
                n = cnt[ch, t]
                e0 = eoff[ch * NT + t]
                slots = tiles[ch, t] * P
                idxs[off:off + n] = row[e0:e0 + n].astype(np.int16)
                dcol[off:off + n] = (d[e0:e0 + n] - t * P).astype(np.float32)
                off += slots
        assert off == TOT * P
        idx_streams.append(idxs)
        dst_streams.append(dcol)

    CALLS = []   # list of (chunk, ntiles) in stream order
    for ch in range(NCH):
        rem = int(tiles_chunk[ch])
        while rem > 0:
            k = min(c["CALL_TILES"], rem)
            CALLS.append((ch, k))
            rem -= k
    idx_wrapped = []
    for ci in range(NCORES):
        stream = idx_streams[ci]
        parts, off = [], 0
        for (_ch, k) in CALLS:
            parts.append(_wrap_idx(stream[off:off + k * P]))
            off += k * P
        idx_wrapped.append(np.concatenate(parts, axis=1))      # [128, TOT*8]
    dst_cols = [ds.reshape(TOT, P).T.copy() for ds in dst_streams]  # [128, TOT]
    return dict(TOT=TOT, tiles=tiles, CALLS=CALLS, idx=idx_wrapped,
                dstc=dst_cols)


def host_prep(cfg, pos, edge_index, batch):
    c = cfg
    N, E0, G, SH = c["N"], c["E0"], c["G"], c["SH"]
    NCH = c["NCHUNK"]
    NCORES = c["NCORES"]
    NPAD = SH * NCORES
    NT = SH // P                      # dst tiles per core
    CH = NPAD // NCH                  # exact chunking
    HSH = SH // 2
    src = np.concatenate([edge_index[0], np.arange(N, dtype=np.int64)]).astype(np.int64)
    dst = np.concatenate([edge_index[1], np.arange(N, dtype=np.int64)]).astype(np.int64)
    deg = np.bincount(dst, minlength=NPAD).astype(np.float32)

    so = np.argsort(dst, kind="stable")
    dsts = dst[so]
    srcs = src[so]
    bounds = np.searchsorted(dsts, np.arange(NCORES + 1) * SH)
    srcs_pc = [srcs[bounds[ci]:bounds[ci + 1]] for ci in range(NCORES)]
    dsts_pc = [dsts[bounds[ci]:bounds[ci + 1]] - ci * SH for ci in range(NCORES)]

    # L1 drops the appended self-loops (their q[d] is added on-device from
    # pos_shard); the E0 random edges keep identity node->row map (qtab).
    so1 = np.argsort(dst[:E0], kind="stable")
    dsts1 = dst[:E0][so1]
    srcs1 = src[:E0][so1]
    bounds1 = np.searchsorted(dsts1, np.arange(NCORES + 1) * SH)
    srcs1_pc = [srcs1[bounds1[ci]:bounds1[ci + 1]] for ci in range(NCORES)]
    dsts1_pc = [dsts1[bounds1[ci]:bounds1[ci + 1]] - ci * SH for ci in range(NCORES)]
    m1 = _edge_meta(cfg, srcs1_pc, dsts1_pc, lambda ci, s: s,
                    np.arange(NCH + 1) * CH)

    # L2: edges from MY OWN nodes read local gshard halves (rows [0, SH));
    # remote edges read the AllGathered halves at SH + core-major-half row
    # (row = h*8*HSH + core*HSH + r%HSH). Chunks: 2 local + 4 remote.
    def rowmap2(ci, s):
        c2 = s // SH
        r = s - c2 * SH
        h = r // HSH
        grow = h * (NCORES * HSH) + c2 * HSH + (r - h * HSH)
        return np.where(c2 == ci, r, SH + grow)
    m2 = _edge_meta(cfg, srcs_pc, dsts_pc, rowmap2,
                    [0, HSH, SH, SH + CH, SH + 2 * CH, SH + 3 * CH, SH + 4 * CH])

    # pooling metadata
    gbase = np.zeros(NCORES, np.int32)
    batch_local = np.full((NCORES, SH), 999.0, np.float32)
    for ci in range(NCORES):
        lo, hi = ci * SH, min((ci + 1) * SH, N)
        gbase[ci] = batch[lo]
        batch_local[ci, :hi - lo] = (batch[lo:hi] - batch[lo]).astype(np.float32)
        assert batch[hi - 1] - batch[lo] < P - 2, "too many graphs in one shard"
    cnts = np.bincount(batch, minlength=G).astype(np.float32)

    meta = dict(
        NPAD=NPAD, NT=NT, CH=CH, m1=m1, m2=m2, NPADROWS=NPAD - N,
    )
    # device-layout inputs (identical shapes across cores; values differ where noted)
    NTF = NPAD // P
    pos_pad = np.zeros((NPAD, c["IN_C"]), np.float32)
    pos_pad[:N] = pos
    pos_dev = pos_pad.reshape(NTF, P, c["IN_C"]).transpose(1, 0, 2).copy()
    pos_shard = [np.ascontiguousarray(
        pos_pad[ci * SH:(ci + 1) * SH].reshape(NT, P, c["IN_C"]).transpose(1, 0, 2))
        for ci in range(NCORES)]
    deg_dev = deg.reshape(NTF, P).T.copy()
    deg_shard = [deg[ci * SH:(ci + 1) * SH].reshape(NT, P).T.copy() for ci in range(NCORES)]
    batch_dev = [batch_local[ci].reshape(NT, P).T.copy() for ci in range(NCORES)]
    cnt_dev = np.zeros((P, (G + P - 1) // P), np.float32)
    for g in range(G):
        cnt_dev[g % P, g // P] = cnts[g]
    ins = []
    for ci in range(NCORES):
        ins.append(dict(
            pos_dev=pos_dev, deg_dev=deg_dev, deg_shard=deg_shard[ci],
            pos_shard=pos_shard[ci],
            idxs1=m1["idx"][ci], dstc1=m1["dstc"][ci],
            idxs2=m2["idx"][ci], dstc2=m2["dstc"][ci],
            batchl=batch_dev[ci], cntg=cnt_dev,
            gbase=np.array([[float(gbase[ci])]], np.float32),
        ))
    return meta, ins


def _prep_weights(cfg, W):
    """Re-layout weights for device (pure replication / transpose-free reshapes)."""
    c = cfg
    HID, OUT, IN_C, G = c["HID"], c["OUT"], c["IN_C"], c["G"]
    NH = HID // P                      # channel halves (2)
    w = {}
    w["w1"] = W["w_conv1"].astype(np.float16)                        # [2, 256]
    w["w1_cols"] = np.ascontiguousarray(
        np.asarray(W["w_conv1"], np.float32).T.reshape(NH, P, IN_C).transpose(1, 0, 2))  # [128, NH, 2]
    w["b1_cols"] = W["b_conv1"].reshape(NH, P).T.astype(np.float32).copy()   # [128, NH]
    w["ln1w_cols"] = W["ln1_w"].reshape(NH, P).T.astype(np.float32).copy()
    w["ln1b_cols"] = W["ln1_b"].reshape(NH, P).T.astype(np.float32).copy()
    w["w2_kt"] = np.ascontiguousarray(
        W["w_conv2"].reshape(NH, P, HID).transpose(1, 0, 2)).astype(np.float16)  # [128, NH, 256]
    w["b2_bc"] = np.tile(W["b_conv2"][None, :], (P, 1)).astype(np.float32)   # [128, 256]
    w["ln2w_bc"] = np.tile(W["ln2_w"][None, :], (P, 1)).astype(np.float32)
    w["ln2b_bc"] = np.tile(W["ln2_b"][None, :], (P, 1)).astype(np.float32)
    w["wl1_kt"] = np.ascontiguousarray(
        W["w_lin1"].reshape(NH, P, HID // 2).transpose(1, 0, 2)).astype(np.float16)  # [128, NH, 128]
    w["bl1_bc"] = np.tile(W["b_lin1"][None, :], (P, 1)).astype(np.float32)   # [128, 128]
    w["lnmw_bc"] = np.tile(W["lnm_w"][None, :], (P, 1)).astype(np.float32)
    w["lnmb_bc"] = np.tile(W["lnm_b"][None, :], (P, 1)).astype(np.float32)
    w["wl2"] = W["w_lin2"].astype(np.float32)                        # [128, 16]
    w["bl2_bc"] = np.tile(W["b_lin2"][None, :], (P, 1)).astype(np.float32)   # [128, 16]
    w["a1"] = float(W["a1"]); w["a2"] = float(W["a2"]); w["am"] = float(W["am"])
    return w


# ----------------------------------------------------------------- device build

def build_program(cfg, meta, weights):
    import concourse.bass as bass
    import concourse.mybir as mybir
    import concourse.tile as tile
    from concourse import bacc
    from concourse.masks import make_identity

    c = cfg
    dt = mybir.dt
    N, G, HID, OUT, IN_C = c["N"], c["G"], c["HID"], c["OUT"], c["IN_C"]
    SH, NCH = c["SH"], c["NCHUNK"]
    NCORES = c["NCORES"]
    NPAD, NT, CH = meta["NPAD"], meta["NT"], meta["CH"]
    em1, em2 = meta["m1"], meta["m2"]
    TOT1, TOT2 = em1["TOT"], em2["TOT"]
    HSH = SH // 2
    NTF = NPAD // P
    NH = HID // P
    GT = (G + P - 1) // P              # graph tiles (4)
    NPADROWS = meta["NPADROWS"]
    EPS = 1e-5
    CORE_IDS = list(range(NCORES))
    f32, f16, i16, i32 = dt.float32, dt.float16, dt.int16, dt.int32
    AF = mybir.ActivationFunctionType
    OP = mybir.AluOpType

    nc = bacc.Bacc("TRN2", debug=False, num_devices=NCORES, num_swdge_queues=4)

    # ---- I/O ----
    pos_in = nc.declare_dram_parameter("pos_dev", [P, NTF, IN_C], f32, isOutput=False)
    deg_in = nc.declare_dram_parameter("deg_dev", [P, NTF], f32, isOutput=False)
    degs_in = nc.declare_dram_parameter("deg_shard", [P, NT], f32, isOutput=False)
    poss_in = nc.declare_dram_parameter("pos_shard", [P, NT, IN_C], f32, isOutput=False)
    idx1_in = nc.declare_dram_parameter("idxs1", [P, TOT1 * 8], i16, isOutput=False)
    dstc1_in = nc.declare_dram_parameter("dstc1", [P, TOT1], f32, isOutput=False)
    idx2_in = nc.declare_dram_parameter("idxs2", [P, TOT2 * 8], i16, isOutput=False)
    dstc2_in = nc.declare_dram_parameter("dstc2", [P, TOT2], f32, isOutput=False)
    batch_in = nc.declare_dram_parameter("batchl", [P, NT], f32, isOutput=False)
    cnt_in = nc.declare_dram_parameter("cntg", [P, GT], f32, isOutput=False)
    gbase_in = nc.declare_dram_parameter("gbase", [1, 1], f32, isOutput=False)
    wt = {}
    wspec = dict(
        w1=([IN_C, HID], f16), w1_cols=([P, NH, IN_C], f32),
        b1_cols=([P, NH], f32), ln1w_cols=([P, NH], f32), ln1b_cols=([P, NH], f32),
        w2_kt=([P, NH, HID], f16), b2_bc=([P, HID], f32),
        ln2w_bc=([P, HID], f32), ln2b_bc=([P, HID], f32),
        wl1_kt=([P, NH, HID // 2], f16), bl1_bc=([P, HID // 2], f32),
        lnmw_bc=([P, HID // 2], f32), lnmb_bc=([P, HID // 2], f32),
        wl2=([HID // 2, OUT], f32), bl2_bc=([P, OUT], f32),
    )
    for k, (shp, dt_) in wspec.items():
        wt[k] = nc.declare_dram_parameter(k, shp, dt_, isOutput=False)
    out_ext = nc.declare_dram_parameter("out", [G, OUT], f32, isOutput=True)

    # ---- internal DRAM ----
    qtab = nc.dram_tensor("qtab", [NPAD, P], f16)                 # L1 table (2 real cols)
    # gshard/gtab split in two halves so the first AllGather overlaps the
    # second half's h2 compute and the L2 gathers of chunks 0-1.
    gshard0 = nc.dram_tensor("gshard0", [HSH, HID], f16)
    gshard1 = nc.dram_tensor("gshard1", [HSH, HID], f16)
    gtab0 = nc.dram_tensor("gtab0", [NCORES * HSH, HID], f16, addr_space="Shared")
    gtab1 = nc.dram_tensor("gtab1", [NCORES * HSH, HID], f16, addr_space="Shared")
    st1_in = nc.dram_tensor("st1_in", [1, P], f32)
    st1_out = nc.dram_tensor("st1_out", [1, P], f32, addr_space="Shared")
    st2_in = nc.dram_tensor("st2_in", [1, P], f32)
    st2_out = nc.dram_tensor("st2_out", [1, P], f32, addr_space="Shared")
    POOLR = GT * P                                                # 512 rows
    pool_in = nc.dram_tensor("pool_in", [POOLR, HID], f16)
    pool_out = nc.dram_tensor("pool_out", [POOLR, HID], f16, addr_space="Shared")

    a1, a2, am = weights["a1"], weights["a2"], weights["am"]

    with tile.TileContext(nc) as tc:
        with tc.tile_pool(name="persist", bufs=1) as pp, \
             tc.tile_pool(name="psc", bufs=4, space="PSUM") as psc:
            # ---- persistent small tiles ----
            iota_i = pp.tile([P, P], i32)
            nc.gpsimd.iota(iota_i[:], pattern=[[1, P]], base=0, channel_multiplier=0)
            iota_h = pp.tile([P, P], f16)
            nc.vector.tensor_copy(out=iota_h[:], in_=iota_i[:])
            iota_f = pp.tile([P, P], f32)
            nc.vector.tensor_copy(out=iota_f[:], in_=iota_i[:])
            ident = pp.tile([P, P], f32)
            make_identity(nc, ident[:])
            ones_col = pp.tile([P, 1], f32)
            nc.vector.memset(ones_col[:], 1.0)
            ones_row = pp.tile([1, P], f32)
            nc.vector.memset(ones_row[:], 1.0)

            # dinv (full + shard)
            deg_f = pp.tile([P, NTF], f32)
            nc.sync.dma_start(out=deg_f[:], in_=deg_in[:])
            nc.vector.tensor_scalar(out=deg_f[:], in0=deg_f[:], scalar1=1.0,
                                    scalar2=None, op0=OP.max)
            nc.scalar.sqrt(deg_f[:], deg_f[:])
            dinv_f = pp.tile([P, NTF], f32)
            nc.vector.reciprocal(dinv_f[:], deg_f[:])
            deg_s = pp.tile([P, NT], f32)
            nc.sync.dma_start(out=deg_s[:], in_=degs_in[:])
            nc.vector.tensor_scalar(out=deg_s[:], in0=deg_s[:], scalar1=1.0,
                                    scalar2=None, op0=OP.max)
            nc.scalar.sqrt(deg_s[:], deg_s[:])
            dinv_s = pp.tile([P, NT], f32)
            nc.vector.reciprocal(dinv_s[:], deg_s[:])

            # ---- build q table: q = pos * dinv (fp16 rows of qtab) ----
            with tc.tile_pool(name="p0", bufs=1) as p0:
                pos_sb = p0.tile([P, NTF, IN_C], f32)
                nc.sync.dma_start(out=pos_sb[:], in_=pos_in[:])
                q16 = p0.tile([P, NTF, IN_C], f16)
                for ch in range(IN_C):
                    nc.vector.tensor_tensor(out=q16[:, :, ch], in0=pos_sb[:, :, ch],
                                            in1=dinv_f[:], op=OP.mult)
                # write per chunk so chunk-0 gathers start before the rest
                qtab_v = qtab[:].rearrange("(a b) d -> b a d", b=P)
                CHT = CH // P
                for chk in range(NCH):
                    nc.sync.dma_start(
                        out=qtab_v[:, chk * CHT:(chk + 1) * CHT, 0:IN_C],
                        in_=q16[:, chk * CHT:(chk + 1) * CHT, :])

            dstc16_1 = pp.tile([P, TOT1], f16)
            dstc16_2 = pp.tile([P, TOT2], f16)
            with tc.tile_pool(name="dstld", bufs=2) as dsp:
                dstc_sb1 = dsp.tile([P, TOT1], f32, tag="d1")
                nc.sync.dma_start(out=dstc_sb1[:], in_=dstc1_in[:])
                nc.vector.tensor_copy(out=dstc16_1[:], in_=dstc_sb1[:])
                dstc_sb2 = dsp.tile([P, TOT2], f32, tag="d2")
                nc.sync.dma_start(out=dstc_sb2[:], in_=dstc2_in[:])
                nc.vector.tensor_copy(out=dstc16_2[:], in_=dstc_sb2[:])

            # zero the pool staging buffer early (independent of everything)
            zero_t = pp.tile([P, HID], f16)
            nc.vector.memset(zero_t[:], 0.0)
            for j in range(GT):
                nc.sync.dma_start(out=pool_in[j * P:(j + 1) * P, :], in_=zero_t[:])

            wsb = {}
            for k, (shp, dt_) in wspec.items():
                wsb[k] = pp.tile(shp, dt_, name=f"w_{k}")
                nc.sync.dma_start(out=wsb[k][:], in_=wt[k][:])

            # helper: cross-partition scalar sum -> [1,1] psum -> sbuf tile
            def part_sum(src_col, w_):
                ps = psc.tile([1, src_col.shape[1]], f32, space="PSUM", tag="psc_scratch")
                nc.tensor.matmul(out=ps[:], lhsT=ones_col[:], rhs=src_col[:],
                                 start=True, stop=True)
                dstt = w_.tile([1, src_col.shape[1]], f32, tag="psum_scalar")
                nc.vector.tensor_copy(out=dstt[:], in_=ps[:])
                return dstt

            def bcast_col(vals_row, w_):
                """vals_row [1, k] -> [128, k] replicated."""
                k = vals_row.shape[1]
                ps = psc.tile([P, k], f32, space="PSUM", tag="psc_scratch")
                nc.tensor.matmul(out=ps[:], lhsT=ones_row[:], rhs=vals_row[:],
                                 start=True, stop=True)
                o = w_.tile([P, k], f32, tag="bcast_col")
                nc.vector.tensor_copy(out=o[:], in_=ps[:])
                return o

            # ============ gather + segsum pass (shared for L1 / L2) ============
            def agg_pass(table_fn, elem, rhs_w, accum, m_, idx_in, dstc16,
                         stage_pool, idxp, selp, psp):
                """table_fn(chk) -> dram region [CH, elem] f16;
                accum [128, NT, rhs_w] zeroed; m_: edge-meta dict."""
                tiles, CALLS = m_["tiles"], m_["CALLS"]
                gt_tile = 0        # global tile index
                call_i = 0
                off_cols = 0       # idx column offset (8 per tile)
                for chk in range(tiles.shape[0]):
                    order = [t for t in range(NT) for _ in range(int(tiles[chk, t]))]
                    # call partitioning for this chunk
                    pos_in_chunk = 0
                    chunk_tiles = int(tiles[chk].sum())
                    while pos_in_chunk < chunk_tiles:
                        k = min(c["CALL_TILES"], chunk_tiles - pos_in_chunk)
                        idx_t = idxp.tile([P, c["CALL_TILES"] * 8], i16, tag="idx")
                        nc.sync.dma_start(
                            out=idx_t[:, 0:k * 8],
                            in_=idx_in[:, off_cols:off_cols + k * 8])
                        gbuf = stage_pool.tile([P, c["CALL_TILES"], elem], f16, tag="g")
                        nc.gpsimd.dma_gather(
                            out_ap=gbuf[:, 0:k, :],
                            in_ap=table_fn(chk),
                            idxs_ap=idx_t[:, 0:k * 8],
                            num_idxs=k * P, num_idxs_reg=k * P,
                            elem_size=elem, single_packet=False,
                            queue_num=call_i % c["QUEUES"])
                        # batched one-hot build for the whole call (one DVE op)
                        sel_call = selp.tile([P, c["CALL_TILES"], P], f16, tag="sel")
                        _i = iota_h[:]
                        _db = dstc16[:, gt_tile:gt_tile + k]
                        iota_bc = bass.AP(_i.tensor, _i.offset,
                                          [list(_i.ap[0]), [0, k], list(_i.ap[1])])
                        dst_bc = bass.AP(_db.tensor, _db.offset,
                                         [list(_db.ap[0]), list(_db.ap[1]), [0, P]])
                        nc.vector.tensor_tensor(out=sel_call[:, 0:k, :], in0=iota_bc,
                                                in1=dst_bc, op=OP.is_equal)
                        # consume the k tiles of this call
                        for j in range(k):
                            t = order[pos_in_chunk + j]
                            first = (order[pos_in_chunk + j - 1] != t) if (pos_in_chunk + j) > 0 else True
                            # new psum group when dtile changes (cells are contiguous)
                            if first:
                                ps = psp.tile([P, rhs_w], f32, space="PSUM", tag="pacc")
                            last = (pos_in_chunk + j == chunk_tiles - 1) or \
                                   (order[pos_in_chunk + j + 1] != t)
                            nc.tensor.matmul(
                                out=ps[:], lhsT=sel_call[:, j, :],
                                rhs=gbuf[:, j, 0:rhs_w],
                                start=first, stop=last)
                            if last:
                                if chk == 0:
                                    nc.vector.tensor_copy(out=accum[:, t, :],
                                                          in_=ps[:])
                                else:
                                    nc.vector.tensor_add(out=accum[:, t, :],
                                                         in0=accum[:, t, :], in1=ps[:])
                            gt_tile += 1
                        pos_in_chunk += k
                        off_cols += k * 8
                        call_i += 1

            # ---- W1/b1 replicated reductions (for moments-based ln1 stats) ----
            # w1aux cols: 0:sum_w0 1:sum_w1 2:sum_w0^2 3:sum_w1^2 4:sum_w0w1
            #             5:sum_b1w0 6:sum_b1w1 7:sum_b1 8:sum_b1^2
            w1aux = pp.tile([1, 10], f32)
            with tc.tile_pool(name="w1a", bufs=1) as wap:
                w0 = wsb["w1_cols"][:, :, 0]
                w1c = wsb["w1_cols"][:, :, 1]
                b1c = wsb["b1_cols"][:]
                wa = wap.tile([P, 9], f32)
                scr = wap.tile([P, NH], f32)

                def _red(dst, src):
                    nc.vector.tensor_reduce(out=dst, in_=src,
                                            axis=mybir.AxisListType.X, op=OP.add)
                _red(wa[:, 0:1], w0)
                _red(wa[:, 1:2], w1c)
                nc.vector.tensor_tensor(out=scr[:], in0=w0, in1=w0, op=OP.mult)
                _red(wa[:, 2:3], scr[:])
                nc.vector.tensor_tensor(out=scr[:], in0=w1c, in1=w1c, op=OP.mult)
                _red(wa[:, 3:4], scr[:])
                nc.vector.tensor_tensor(out=scr[:], in0=w0, in1=w1c, op=OP.mult)
                _red(wa[:, 4:5], scr[:])
                nc.vector.tensor_tensor(out=scr[:], in0=b1c, in1=w0, op=OP.mult)
                _red(wa[:, 5:6], scr[:])
                nc.vector.tensor_tensor(out=scr[:], in0=b1c, in1=w1c, op=OP.mult)
                _red(wa[:, 6:7], scr[:])
                _red(wa[:, 7:8], b1c)
                nc.vector.tensor_tensor(out=scr[:], in0=b1c, in1=b1c, op=OP.mult)
                _red(wa[:, 8:9], scr[:])
                t9 = part_sum(wa, wap)
                nc.vector.tensor_copy(out=w1aux[:, 0:9], in_=t9[:])

            # =============================== L1 ===============================
            with tc.tile_pool(name="acc1", bufs=1) as a1p:
                accum1 = a1p.tile([P, NT, IN_C], f32)
                with tc.tile_pool(name="st1", bufs=8) as sp1, \
                     tc.tile_pool(name="idx1", bufs=8) as ip1, \
                     tc.tile_pool(name="sel1", bufs=6) as sl1, \
                     tc.tile_pool(name="ps1", bufs=3, space="PSUM") as pp1, \
                     nc.named_scope("L1agg"):
                    agg_pass(lambda chk: qtab[chk * CH:(chk + 1) * CH, :],
                             P, IN_C, accum1, em1, idx1_in, dstc16_1,
                             sp1, ip1, sl1, pp1)

                # ---- ln1 stats from moments of s = dinv*agg (no x1 pass) ----
                with tc.tile_pool(name="x1", bufs=1) as x1p, nc.named_scope("mid"):
                    x1t = x1p.tile([P, NH, SH], f16)
                    acol = x1p.tile([P, NH], f32)
                    bcol = x1p.tile([P, NH], f32)
                    _dv = dinv_s[:]
                    dinv_bc2 = bass.AP(_dv.tensor, _dv.offset,
                                       [list(_dv.ap[0]), list(_dv.ap[1]), [0, IN_C]])
                    # add the self-loop term q[d] = pos[d]*dinv[d] locally
                    # (L1 edge streams exclude the appended self-loops)
                    poss_sb = x1p.tile([P, NT, IN_C], f32)
                    nc.sync.dma_start(out=poss_sb[:], in_=poss_in[:])
                    selfq = x1p.tile([P, NT, IN_C], f32)
                    nc.vector.tensor_tensor(out=selfq[:], in0=poss_sb[:],
                                            in1=dinv_bc2, op=OP.mult)
                    nc.vector.tensor_add(out=accum1[:], in0=accum1[:], in1=selfq[:])
                    sct = x1p.tile([P, NT, IN_C], f16)
                    nc.vector.tensor_tensor(out=sct[:], in0=accum1[:], in1=dinv_bc2,
                                            op=OP.mult)
                    with tc.tile_pool(name="st1p", bufs=1) as stp:
                        pr = stp.tile([P, NT], f16)
                        cols = stp.tile([P, 5], f32)
                        nc.vector.tensor_reduce(out=cols[:, 0:1], in_=sct[:, :, 0],
                                                axis=mybir.AxisListType.X, op=OP.add)
                        nc.vector.tensor_reduce(out=cols[:, 1:2], in_=sct[:, :, 1],
                                                axis=mybir.AxisListType.X, op=OP.add)
                        nc.vector.tensor_tensor(out=pr[:], in0=sct[:, :, 0],
                                                in1=sct[:, :, 0], op=OP.mult)
                        nc.vector.tensor_reduce(out=cols[:, 2:3], in_=pr[:],
                                                axis=mybir.AxisListType.X, op=OP.add)
                        nc.vector.tensor_tensor(out=pr[:], in0=sct[:, :, 1],
                                                in1=sct[:, :, 1], op=OP.mult)
                        nc.vector.tensor_reduce(out=cols[:, 3:4], in_=pr[:],
                                                axis=mybir.AxisListType.X, op=OP.add)
                        nc.vector.tensor_tensor(out=pr[:], in0=sct[:, :, 0],
                                                in1=sct[:, :, 1], op=OP.mult)
                        nc.vector.tensor_reduce(out=cols[:, 4:5], in_=pr[:],
                                                axis=mybir.AxisListType.X, op=OP.add)
                        tot = part_sum(cols, stp)            # [1,5] local S0,S1,M00,M11,M01
                        arr = stp.tile([1, P], f32)
                        nc.vector.memset(arr[:], 0.0)
                        nc.vector.tensor_copy(out=arr[:, 0:5], in_=tot[:])
                        nc.sync.dma_start(out=st1_in[:], in_=arr[:])
                        nc.gpsimd.collective_compute(
                            "AllReduce", OP.add, replica_groups=[CORE_IDS],
                            ins=[st1_in[:]], outs=[st1_out[:]])
                        arro = stp.tile([1, P], f32)
                        nc.sync.dma_start(out=arro[:], in_=st1_out[:])
                        # sumx = S.w1sum + N*b1sum ; sumsq = quad(M,w1) + 2*S.b1w + N*b1sq
                        CNT = float(N * HID)
                        s2 = stp.tile([1, 2], f32)
                        nc.vector.tensor_tensor(out=s2[:], in0=arro[:, 0:2],
                                                in1=w1aux[:, 0:2], op=OP.mult)
                        sumx = stp.tile([1, 1], f32)
                        nc.vector.tensor_add(out=sumx[:], in0=s2[:, 0:1], in1=s2[:, 1:2])
                        t1 = stp.tile([1, 1], f32)
                        nc.vector.tensor_scalar(out=t1[:], in0=w1aux[:, 7:8],
                                                scalar1=float(N), scalar2=None, op0=OP.mult)
                        nc.vector.tensor_add(out=sumx[:], in0=sumx[:], in1=t1[:])
                        q3 = stp.tile([1, 3], f32)
                        nc.vector.tensor_tensor(out=q3[:], in0=arro[:, 2:5],
                                                in1=w1aux[:, 2:5], op=OP.mult)
                        sb2 = stp.tile([1, 2], f32)
                        nc.vector.tensor_tensor(out=sb2[:], in0=arro[:, 0:2],
                                                in1=w1aux[:, 5:7], op=OP.mult)
                        sumsq = stp.tile([1, 1], f32)
                        nc.vector.tensor_add(out=sumsq[:], in0=q3[:, 0:1], in1=q3[:, 1:2])
                        nc.vector.tensor_scalar(out=t1[:], in0=q3[:, 2:3],
                                                scalar1=2.0, scalar2=None, op0=OP.mult)
                        nc.vector.tensor_add(out=sumsq[:], in0=sumsq[:], in1=t1[:])
                        nc.vector.tensor_add(out=t1[:], in0=sb2[:, 0:1], in1=sb2[:, 1:2])
                        nc.vector.tensor_scalar(out=t1[:], in0=t1[:],
                                                scalar1=2.0, scalar2=None, op0=OP.mult)
                        nc.vector.tensor_add(out=sumsq[:], in0=sumsq[:], in1=t1[:])
                        nc.vector.tensor_scalar(out=t1[:], in0=w1aux[:, 8:9],
                                                scalar1=float(N), scalar2=None, op0=OP.mult)
                        nc.vector.tensor_add(out=sumsq[:], in0=sumsq[:], in1=t1[:])
                        mean_t = stp.tile([1, 1], f32)
                        nc.vector.tensor_scalar(out=mean_t[:], in0=sumx[:],
                                                scalar1=1.0 / CNT, scalar2=None, op0=OP.mult)
                        ex2 = stp.tile([1, 1], f32)
                        nc.vector.tensor_scalar(out=ex2[:], in0=sumsq[:],
                                                scalar1=1.0 / CNT, scalar2=None, op0=OP.mult)
                        m2 = stp.tile([1, 1], f32)
                        nc.vector.tensor_tensor(out=m2[:], in0=mean_t[:], in1=mean_t[:],
                                                op=OP.mult)
                        var = stp.tile([1, 1], f32)
                        nc.vector.tensor_tensor(out=var[:], in0=ex2[:], in1=m2[:],
                                                op=OP.subtract)
                        nc.scalar.sqrt(var[:], var[:])
                        nc.vector.tensor_scalar(out=var[:], in0=var[:], scalar1=EPS,
                                                scalar2=None, op0=OP.add)
                        rstd = stp.tile([1, 1], f32)
                        nc.vector.reciprocal(rstd[:], var[:])
                        pack = stp.tile([1, 2], f32)
                        nc.vector.tensor_copy(out=pack[:, 0:1], in_=mean_t[:])
                        nc.vector.tensor_copy(out=pack[:, 1:2], in_=rstd[:])
                        mr = bcast_col(pack, stp)            # [128,2] (mean, rstd)
                        # x1' = (x1-mu)*rstd*w + b = x1*acol + bcol (per-channel)
                        nc.vector.tensor_scalar(out=acol[:], in0=wsb["ln1w_cols"][:],
                                                scalar1=mr[:, 1:2], scalar2=None, op0=OP.mult)
                        nc.vector.tensor_scalar(out=bcol[:], in0=wsb["b1_cols"][:],
                                                scalar1=mr[:, 0:1], scalar2=None,
                                                op0=OP.subtract)
                        nc.vector.tensor_tensor(out=bcol[:], in0=bcol[:], in1=acol[:],
                                                op=OP.mult)
                        nc.vector.tensor_add(out=bcol[:], in0=bcol[:],
                                             in1=wsb["ln1b_cols"][:])

                    # ---- fused per tile: x1 = prelu(a*(W1@s.T)+b) then
                    #      h2 = x1' @ W2; g = dinv*h2 -> gshard halves ----
                    with tc.tile_pool(name="w2w", bufs=6) as wk, \
                         tc.tile_pool(name="px1", bufs=2, space="PSUM") as px1, \
                         tc.tile_pool(name="h2w", bufs=4) as h2w, \
                         tc.tile_pool(name="ph2", bufs=2, space="PSUM") as ph2:
                        for t in range(NT):
                            sc = wk.tile([P, IN_C], f32, tag="sc")
                            nc.vector.tensor_scalar(
                                out=sc[:], in0=accum1[:, t, :],
                                scalar1=dinv_s[:, t:t + 1], scalar2=None, op0=OP.mult)
                            pt = psc.tile([IN_C, P], f32, space="PSUM", tag="psc_scratch")
                            nc.tensor.transpose(out=pt[:], in_=sc[:], identity=ident[:])
                            p1t = wk.tile([IN_C, P], f16, tag="p1t_sb")
                            nc.vector.tensor_copy(out=p1t[:], in_=pt[:])
                            for h in range(NH):
                                psx = px1.tile([P, P], f32, space="PSUM", tag="px1")
                                nc.tensor.matmul(
                                    out=psx[:], lhsT=wsb["w1"][:, h * P:(h + 1) * P],
                                    rhs=p1t[:], start=True, stop=True)
                                nc.scalar.activation(
                                    out=x1t[:, h, t * P:(t + 1) * P], in_=psx[:],
                                    func=AF.Prelu, bias=bcol[:, h:h + 1],
                                    scale=acol[:, h:h + 1], alpha=a1)
                            ps2 = ph2.tile([P, HID], f32, space="PSUM", tag="ph2")
                            for h in range(NH):
                                nc.tensor.matmul(
                                    out=ps2[:], lhsT=x1t[:, h, t * P:(t + 1) * P],
                                    rhs=wsb["w2_kt"][:, h, :], start=(h == 0), stop=(h == NH - 1))
                            g16 = h2w.tile([P, HID], f16, tag="g16")
                            nc.vector.tensor_scalar(
                                out=g16[:], in0=ps2[:],
                                scalar1=dinv_s[:, t:t + 1], scalar2=None, op0=OP.mult)
                            if t < NT // 2:
                                nc.sync.dma_start(
                                    out=gshard0[t * P:(t + 1) * P, :], in_=g16[:])
                            else:
                                t_ = t - NT // 2
                                nc.sync.dma_start(
                                    out=gshard1[t_ * P:(t_ + 1) * P, :], in_=g16[:])
            # pools a1p/x1p closed
            with nc.named_scope("allgather"):
                nc.gpsimd.collective_compute(
                    "AllGather", OP.bypass, replica_groups=[CORE_IDS],
                    ins=[gshard0[:]], outs=[gtab0[:]])
                nc.gpsimd.collective_compute(
                    "AllGather", OP.bypass, replica_groups=[CORE_IDS],
                    ins=[gshard1[:]], outs=[gtab1[:]])

            # =============================== L2 ===============================
            with tc.tile_pool(name="acc2", bufs=1) as a2p:
                accum2 = a2p.tile([P, NT, HID], f16)

                def l2_table(chk):
                    if chk == 0:
                        return gshard0[:]
                    if chk == 1:
                        return gshard1[:]
                    gt_ = gtab0 if chk < 4 else gtab1
                    base = (chk - 2) % 2
                    return gt_[base * CH:(base + 1) * CH, :]
                with tc.tile_pool(name="st2", bufs=5) as sp2, \
                     tc.tile_pool(name="idx2", bufs=8) as ip2, \
                     tc.tile_pool(name="sel2", bufs=4) as sl2, \
                     tc.tile_pool(name="ps2", bufs=4, space="PSUM") as pp2, \
                     nc.named_scope("L2agg"):
                    agg_pass(l2_table, HID, HID, accum2, em2, idx2_in, dstc16_2,
                             sp2, ip2, sl2, pp2)

                # ---- x2 = dinv*agg + b2 per-tile (pipelines into L2agg tail),
                #      ln2 stats, prelu ----
                with tc.tile_pool(name="stats2", bufs=1) as stp:
                    b2_16 = stp.tile([P, HID], f16)
                    nc.vector.tensor_copy(out=b2_16[:], in_=wsb["b2_bc"][:])
                    s_cols2 = stp.tile([P, NT], f32)
                    sq_cols2 = stp.tile([P, NT], f32)
                    with tc.tile_pool(name="sqscr", bufs=4) as sqp:
                        for t in range(NT):
                            nc.vector.tensor_scalar(
                                out=accum2[:, t, :], in0=accum2[:, t, :],
                                scalar1=dinv_s[:, t:t + 1], scalar2=None,
                                op0=OP.mult)
                            nc.vector.tensor_tensor(out=accum2[:, t, :],
                                                    in0=accum2[:, t, :],
                                                    in1=b2_16[:], op=OP.add)
                            sq_scr2 = sqp.tile([P, HID], f16, tag="sqs")
                            nc.scalar.activation(out=sq_scr2[:], in_=accum2[:, t, :],
                                                 func=AF.Square,
                                                 accum_out=sq_cols2[:, t:t + 1])
                            nc.vector.tensor_reduce(out=s_cols2[:, t:t + 1],
                                                    in_=accum2[:, t, :],
                                                    axis=mybir.AxisListType.X,
                                                    op=OP.add)
                    s_col = stp.tile([P, 1], f32)
                    nc.vector.tensor_reduce(out=s_col[:], in_=s_cols2[:],
                                            axis=mybir.AxisListType.X, op=OP.add)
                    sq_col = stp.tile([P, 1], f32)
                    nc.vector.tensor_reduce(out=sq_col[:], in_=sq_cols2[:],
                                            axis=mybir.AxisListType.X, op=OP.add)
                    both = stp.tile([P, 2], f32)
                    nc.vector.tensor_copy(out=both[:, 0:1], in_=s_col[:])
                    nc.vector.tensor_copy(out=both[:, 1:2], in_=sq_col[:])
                    tot = part_sum(both, stp)
                    # b2 pad corrections (pad rows equal b2 exactly)
                    b2p = stp.tile([1, 2], f32)
                    nc.vector.tensor_reduce(out=b2p[:, 0:1], in_=wsb["b2_bc"][0:1, :],
                                            axis=mybir.AxisListType.X, op=OP.add)
                    b2sq = stp.tile([1, HID], f32)
                    nc.scalar.square(b2sq[:], wsb["b2_bc"][0:1, :])
                    nc.vector.tensor_reduce(out=b2p[:, 1:2], in_=b2sq[:],
                                            axis=mybir.AxisListType.X, op=OP.add)
                    arr = stp.tile([1, P], f32)
                    nc.vector.memset(arr[:], 0.0)
                    nc.vector.tensor_copy(out=arr[:, 0:2], in_=tot[:])
                    nc.sync.dma_start(out=st2_in[:], in_=arr[:])
                    nc.gpsimd.collective_compute(
                        "AllReduce", OP.add, replica_groups=[CORE_IDS],
                        ins=[st2_in[:]], outs=[st2_out[:]])
                    arro = stp.tile([1, P], f32)
                    nc.sync.dma_start(out=arro[:], in_=st2_out[:])
                    CNT = float(N * HID)
                    cor = stp.tile([1, 2], f32)
                    nc.vector.tensor_scalar(out=cor[:], in0=b2p[:],
                                            scalar1=-float(NPADROWS), scalar2=None, op0=OP.mult)
                    nc.vector.tensor_add(out=cor[:], in0=cor[:], in1=arro[:, 0:2])
                    mean_t = stp.tile([1, 1], f32)
                    nc.vector.tensor_scalar(out=mean_t[:], in0=cor[:, 0:1],
                                            scalar1=1.0 / CNT, scalar2=None, op0=OP.mult)
                    ex2 = stp.tile([1, 1], f32)
                    nc.vector.tensor_scalar(out=ex2[:], in0=cor[:, 1:2],
                                            scalar1=1.0 / CNT, scalar2=None, op0=OP.mult)
                    m2 = stp.tile([1, 1], f32)
                    nc.vector.tensor_tensor(out=m2[:], in0=mean_t[:], in1=mean_t[:], op=OP.mult)
                    var = stp.tile([1, 1], f32)
                    nc.vector.tensor_tensor(out=var[:], in0=ex2[:], in1=m2[:], op=OP.subtract)
                    nc.scalar.sqrt(var[:], var[:])
                    nc.vector.tensor_scalar(out=var[:], in0=var[:], scalar1=EPS,
                                            scalar2=None, op0=OP.add)
                    rstd = stp.tile([1, 1], f32)
                    nc.vector.reciprocal(rstd[:], var[:])
                    pack = stp.tile([1, 2], f32)
                    nc.vector.tensor_copy(out=pack[:, 0:1], in_=mean_t[:])
                    nc.vector.tensor_copy(out=pack[:, 1:2], in_=rstd[:])
                    mr = bcast_col(pack, stp)
                    a_bc = stp.tile([P, HID], f32)
                    c_bc = stp.tile([P, HID], f32)
                    nc.vector.tensor_scalar(out=a_bc[:], in0=wsb["ln2w_bc"][:],
                                            scalar1=mr[:, 1:2], scalar2=None, op0=OP.mult)
                    nc.vector.tensor_scalar(out=c_bc[:], in0=a_bc[:],
                                            scalar1=mr[:, 0:1], scalar2=None, op0=OP.mult)
                    nc.vector.tensor_tensor(out=c_bc[:], in0=wsb["ln2b_bc"][:],
                                            in1=c_bc[:], op=OP.subtract)
                    a_16 = a2p.tile([P, HID], f16)
                    c_16 = a2p.tile([P, HID], f16)
                    nc.vector.tensor_copy(out=a_16[:], in_=a_bc[:])
                    nc.vector.tensor_copy(out=c_16[:], in_=c_bc[:])

                # =========================== pooling ===========================
                with tc.tile_pool(name="poolp", bufs=1) as plp, \
                     tc.tile_pool(name="pps", bufs=1, space="PSUM") as pps, \
                     nc.named_scope("tail"):
                    batch_sb = plp.tile([P, NT], f32)
                    nc.sync.dma_start(out=batch_sb[:], in_=batch_in[:])
                    psg = pps.tile([P, HID], f32, space="PSUM", tag="psg")
                    with tc.tile_pool(name="selg", bufs=6) as slg:
                        for t in range(NT):
                            nc.vector.tensor_tensor(out=accum2[:, t, :],
                                                    in0=accum2[:, t, :],
                                                    in1=a_16[:], op=OP.mult)
                            nc.vector.tensor_tensor(out=accum2[:, t, :],
                                                    in0=accum2[:, t, :],
                                                    in1=c_16[:], op=OP.add)
                            nc.scalar.activation(out=accum2[:, t, :],
                                                 in_=accum2[:, t, :],
                                                 func=AF.Prelu, alpha=a2)
                            selg = slg.tile([P, P], f16, tag="selg")
                            nc.vector.tensor_scalar(
                                out=selg[:], in0=iota_h[:],
                                scalar1=batch_sb[:, t:t + 1], scalar2=None,
                                op0=OP.is_equal)
                            nc.tensor.matmul(out=psg[:], lhsT=selg[:],
                                             rhs=accum2[:, t, :],
                                             start=(t == 0), stop=(t == NT - 1))
                    partial = plp.tile([P, HID], f16)
                    nc.vector.tensor_copy(out=partial[:], in_=psg[:])
                    # place rows at graph_base via one-hot matmuls; zero the rest
                    gb_sb = plp.tile([1, 1], f32)
                    nc.sync.dma_start(out=gb_sb[:], in_=gbase_in[:])
                    gb_col = bcast_col(gb_sb, plp)           # [128,1]
                    loc_col = plp.tile([P, 1], f32)
                    nc.vector.tensor_copy(out=loc_col[:], in_=iota_f[:, 0:1])
                    # iota column: iota_f[:,0:1] is all zeros (values are along free dim)
                    # need per-partition index 0..127: use iota with channel_multiplier=1
                    pidx_i = plp.tile([P, 1], i32)
                    nc.gpsimd.iota(pidx_i[:], pattern=[[0, 1]], base=0, channel_multiplier=1)
                    pidx = plp.tile([P, 1], f32)
                    nc.vector.tensor_copy(out=pidx[:], in_=pidx_i[:])
                    nc.vector.tensor_add(out=loc_col[:], in0=pidx[:], in1=gb_col[:])
                    with tc.tile_pool(name="plc", bufs=3) as plc, \
                         tc.tile_pool(name="ppl", bufs=2, space="PSUM") as ppl:
                        for j in range(GT):
                            sh_col = plc.tile([P, 1], f32, tag="shc")
                            nc.vector.tensor_scalar(out=sh_col[:], in0=loc_col[:],
                                                    scalar1=-float(j * P), scalar2=None,
                                                    op0=OP.add)
                            selj = plc.tile([P, P], f16, tag="selj")
                            nc.vector.tensor_scalar(out=selj[:], in0=iota_h[:],
                                                    scalar1=sh_col[:], scalar2=None,
                                                    op0=OP.is_equal)
                            psj = ppl.tile([P, HID], f32, space="PSUM", tag="psj")
                            nc.tensor.matmul(out=psj[:], lhsT=selj[:], rhs=partial[:],
                                             start=True, stop=True)
                            oj = plc.tile([P, HID], f16, tag="oj")
                            nc.vector.tensor_copy(out=oj[:], in_=psj[:])
                            nc.sync.dma_start(out=pool_in[j * P:(j + 1) * P, :], in_=oj[:])
                    nc.gpsimd.collective_compute(
                        "AllReduce", OP.add, replica_groups=[CORE_IDS],
                        ins=[pool_in[:]], outs=[pool_out[:]])

                    # ---- head (redundant on every core) ----
                    cnt_sb = plp.tile([P, GT], f32)
                    nc.sync.dma_start(out=cnt_sb[:], in_=cnt_in[:])
                    nc.vector.tensor_scalar(out=cnt_sb[:], in0=cnt_sb[:], scalar1=1.0,
                                            scalar2=None, op0=OP.max)
                    rec_sb = plp.tile([P, GT], f32)
                    nc.vector.reciprocal(rec_sb[:], cnt_sb[:])
                    pooled16r = plp.tile([P, GT, HID], f16)
                    nc.sync.dma_start(
                        out=pooled16r[:],
                        in_=pool_out[0:G, :].rearrange("(a b) d -> b a d", b=P))
                    pooled = plp.tile([P, GT, HID], f32)
                    for j in range(GT):
                        nc.vector.tensor_scalar(out=pooled[:, j, :],
                                                in0=pooled16r[:, j, :],
                                                scalar1=rec_sb[:, j:j + 1], scalar2=None,
                                                op0=OP.mult)
                    # pooledT [128ch, NH, G] (f16 for the head matmuls)
                    pooledT = plp.tile([P, NH, G], f16)
                    with tc.tile_pool(name="trp", bufs=2) as trp:
                        for j in range(GT):
                            for h in range(NH):
                                ptp = psc.tile([P, P], f32, space="PSUM", tag="psc_scratch")
                                nc.tensor.transpose(
                                    out=ptp[:], in_=pooled[:, j, h * P:(h + 1) * P],
                                    identity=ident[:])
                                nc.vector.tensor_copy(
                                    out=pooledT[:, h, j * P:(j + 1) * P], in_=ptp[:])
                    # h1 = pooled @ Wl1 + bl1 : [G, 128]
                    HW = HID // 2
                    h1 = plp.tile([P, GT, HW], f32)
                    with tc.tile_pool(name="ph1", bufs=2, space="PSUM") as ph1:
                        for j in range(GT):
                            psh = ph1.tile([P, HW], f32, space="PSUM", tag="psh")
                            for h in range(NH):
                                nc.tensor.matmul(
                                    out=psh[:], lhsT=pooledT[:, h, j * P:(j + 1) * P],
                                    rhs=wsb["wl1_kt"][:, h, :], start=(h == 0), stop=(h == NH - 1))
                            nc.vector.tensor_add(out=h1[:, j, :], in0=psh[:],
                                                 in1=wsb["bl1_bc"][:])
                    # lnm (local, exact: G*HW elements)
                    s_col = plp.tile([P, 1], f32)
                    nc.vector.tensor_reduce(out=s_col[:], in_=h1[:].rearrange("p a b -> p (a b)"),
                                            axis=mybir.AxisListType.X, op=OP.add)
                    sq_col = plp.tile([P, 1], f32)
                    sqt2 = plp.tile([P, GT * HW], f32)
                    nc.scalar.activation(out=sqt2[:], in_=h1[:].rearrange("p a b -> p (a b)"),
                                         func=AF.Square, accum_out=sq_col[:])
                    both = plp.tile([P, 2], f32)
                    nc.vector.tensor_copy(out=both[:, 0:1], in_=s_col[:])
                    nc.vector.tensor_copy(out=both[:, 1:2], in_=sq_col[:])
                    tot = part_sum(both, plp)
                    CNTM = float(G * HW)
                    mean_t = plp.tile([1, 1], f32)
                    nc.vector.tensor_scalar(out=mean_t[:], in0=tot[:, 0:1],
                                            scalar1=1.0 / CNTM, scalar2=None, op0=OP.mult)
                    ex2 = plp.tile([1, 1], f32)
                    nc.vector.tensor_scalar(out=ex2[:], in0=tot[:, 1:2],
                                            scalar1=1.0 / CNTM, scalar2=None, op0=OP.mult)
                    m2 = plp.tile([1, 1], f32)
                    nc.vector.tensor_tensor(out=m2[:], in0=mean_t[:], in1=mean_t[:], op=OP.mult)
                    var = plp.tile([1, 1], f32)
                    nc.vector.tensor_tensor(out=var[:], in0=ex2[:], in1=m2[:], op=OP.subtract)
                    nc.scalar.sqrt(var[:], var[:])
                    nc.vector.tensor_scalar(out=var[:], in0=var[:], scalar1=EPS,
                                            scalar2=None, op0=OP.add)
                    rstd = plp.tile([1, 1], f32)
                    nc.vector.reciprocal(rstd[:], var[:])
                    pack = plp.tile([1, 2], f32)
                    nc.vector.tensor_copy(out=pack[:, 0:1], in_=mean_t[:])
                    nc.vector.tensor_copy(out=pack[:, 1:2], in_=rstd[:])
                    mr = bcast_col(pack, plp)
                    a_bc = plp.tile([P, HW], f32)
                    c_bc = plp.tile([P, HW], f32)
                    nc.vector.tensor_scalar(out=a_bc[:], in0=wsb["lnmw_bc"][:],
                                            scalar1=mr[:, 1:2], scalar2=None, op0=OP.mult)
                    nc.vector.tensor_scalar(out=c_bc[:], in0=a_bc[:],
                                            scalar1=mr[:, 0:1], scalar2=None, op0=OP.mult)
                    nc.vector.tensor_tensor(out=c_bc[:], in0=wsb["lnmb_bc"][:],
                                            in1=c_bc[:], op=OP.subtract)
                    for j in range(GT):
                        nc.vector.tensor_tensor(out=h1[:, j, :], in0=h1[:, j, :],
                                                in1=a_bc[:], op=OP.mult)
                        nc.vector.tensor_add(out=h1[:, j, :], in0=h1[:, j, :], in1=c_bc[:])
                    nc.scalar.activation(out=h1[:].rearrange("p a b -> p (a b)"),
                                         in_=h1[:].rearrange("p a b -> p (a b)"),
                                         func=AF.Prelu, alpha=am)
                    # out = h1' @ wl2 + bl2
                    outt = plp.tile([P, GT, OUT], f32)
                    with tc.tile_pool(name="of", bufs=2) as ofp:
                        for j in range(GT):
                            ptp = psc.tile([P, P], f32, space="PSUM", tag="psc_scratch")
                            nc.tensor.transpose(out=ptp[:], in_=h1[:, j, :],
                                                identity=ident[:])
                            h1t = ofp.tile([P, P], f32, tag="h1t")
                            nc.vector.tensor_copy(out=h1t[:], in_=ptp[:])
                            pso = psc.tile([P, OUT], f32, space="PSUM", tag="psc_scratch")
                            nc.tensor.matmul(out=pso[:], lhsT=h1t[:], rhs=wsb["wl2"][:],
                                             start=True, stop=True)
                            nc.vector.tensor_add(out=outt[:, j, :], in0=pso[:],
                                                 in1=wsb["bl2_bc"][:, 0:OUT])
                    nc.sync.dma_start(
                        out=out_ext[:].rearrange("(a b) d -> b a d", b=P),
                        in_=outt[:])

    nc.compile()
    return nc


# ----------------------------------------------------------------- entry point

def _run(cfg, inputs, use_sim=False):
    import sys
    if '/opt/trn_rl_repo' not in sys.path:
        sys.path.insert(0, '/opt/trn_rl_repo')
    pos = np.asarray(inputs["pos"], np.float32)
    ei = np.asarray(inputs["edge_index"], np.int64)
    batch = np.asarray(inputs["batch"], np.int64)
    meta, core_ins = host_prep(cfg, pos, ei, batch)
    w = _prep_weights(cfg, inputs)
    nc = build_program(cfg, meta, w)
    for ci in range(cfg["NCORES"]):
        for k, v in w.items():
            if isinstance(v, np.ndarray):
                core_ins[ci][k] = v
    if use_sim:
        from concourse.bass_interp import MultiCoreSim
        sim = MultiCoreSim(nc, cfg["NCORES"], require_finite=False,
                           require_nnan=False)
        for ci in range(cfg["NCORES"]):
            for k, v in core_ins[ci].items():
                sim.cores[ci].tensor(k)[:] = v
        sim.simulate()
        return np.array(sim.cores[0].tensor("out")), None
    from concourse.bass_utils import run_bass_kernel_spmd
    res = run_bass_kernel_spmd(nc, core_ins, list(range(cfg["NCORES"])))
    return res.results[0]["out"], res


def kernel(**inputs):
    out, _ = _run(_cfg_full(), inputs)
    return out

